# revision 13
# baseline (speedup 1.0000x reference)
"""DeepSpeed-style self-attention block on 8 Trainium2 NeuronCores.

Tensor-parallel over heads (4 heads/core), DeepSpeed mp_size=8 style:
  - w_qkv column-sharded [H, 3H/8] (per-core wq/wk/wv [H, 512], bf16)
  - w_out row-sharded   [H/8, H]  -> per-core partial outputs
  - layernorm stats computed on device; partial-sum reduction + b_out on host.

Device kernel structure (per core, identical SPMD program, sharded inputs):
  Phase A: the host passes x twice in bf16 - row-major (for LN stats) and
    pre-transposed xT [H, T] (pure layout change).  Per 512-token chunk:
    DVE bn_stats/bn_aggr give mean/rstd/std columns; a tiny PE matmul
    transposes them to rows.  The QKV gemms run directly on RAW xT
    (bf16, full rate); layernorm is folded in algebraically:
       qT = ((W^T xT) - csum(W) (x) mu + b (x) std) * rstd[t]
    The two rank-1 terms are extra K=1 matmuls accumulated into the same
    PSUM; the per-token rstd scale is one DVE multiply against a
    PE-broadcast rstd row (q/k outputs, free-dim tokens) or a per-partition
    tensor_scalar (v output, partition-dim tokens).  norm_w / the
    1/sqrt(sqrt(hd)) scale are folded into the bf16 weights on the host;
    column sums are taken over the *rounded* bf16 weights so the mean
    correction is exact.  Weights stream as one 4MB slab DMA per
    (weight, chunk).  qT/kT [d, tok] and v [tok, d] go to DRAM scratch in
    bf16.
  Phase B: per (batch, head): scoresT = kT^T @ qT -> [k, q] psum blocks
    (two 512-wide kj blocks per 2-bank psum tile); causal handled as
    additive -50 tiles on the diagonal-straddling blocks only (DVE);
    exp via ACT with per-key (mask+alibi) bias; pairs of e tiles are
    pre-added on DVE so the row-sum ones-matmul runs once per 2 kj;
    PV matmul accumulates ctxT [d, q]; 1/sum broadcast via K=1 matmul,
    applied on the PSUM->SBUF ctx write (DVE).
    Rows that DeepSpeed's tied -10000 constants make attend over the
    *whole* sequence (all their visible keys input-masked) can only live
    among the first <128 queries (host-verified); a narrow pass-2 covers
    queries 0..127 over kj 4..15 with multiplicative key weights
    m2 = exp(alibi + maskbias + NEG), accumulated into the same psum.
    (-50 instead of DeepSpeed's -10000 keeps exp() in fp32 range; softmax
    is shift-invariant and masked weights come out ~1e-19, matching the
    reference's exact 0s well below tolerance.)
  Phase C: out-proj partial: out += ctxT^T @ w_out_shard per token tile.

All big matmuls run in bf16 (full rate); accumulation is f32 PSUM.
The walrus build here allows only ONE semaphore wait per instruction;
PatchedTileContext splits surplus Tile-emitted waits onto NoOps.
"""

import numpy as np
import ml_dtypes

import concourse.bass as bass
import concourse.mybir as mybir
import concourse.tile as tile
from concourse import masks

f32 = mybir.dt.float32
f32r = mybir.dt.float32r
bf16 = mybir.dt.bfloat16
BF = ml_dtypes.bfloat16

B, S, H, NH = 2, 2048, 4096, 32
HD = H // NH            # 128 head dim
NCORES = 8
HPC = NH // NCORES      # 4 heads per core
FPC = HPC * HD          # 512 sharded features per core
T = B * S               # 4096 tokens
KT = H // 128           # 32 contraction tiles
CHUNK = 512             # tokens per QKV chunk
NCHUNK = T // CHUNK     # 8
QTILE = 512             # query block in attention
NP2 = 12                # pass-2 kj blocks (kj 4..15) for fully-masked rows
LN_EPS = 1e-5
NEG = -50.0             # soft mask value (see module docstring)


class PatchedTileContext(tile.TileContext):
    """This container's walrus build rejects >1 sync-wait per instruction;
    split surplus waits onto preceding same-engine NoOps."""

    _wsplit_n = 0

    def _commit_instruction(self, inst, lazy_reg_writes: bool = True):
        si = inst.sync_info
        if si is not None and si.on_wait and len(si.on_wait) > 1:
            waits = list(si.on_wait)
            inst.sync_info = mybir.SyncInfo(
                on_wait=[waits[-1]], on_update=list(si.on_update or [])
            )
            for w in waits[:-1]:
                type(self)._wsplit_n += 1
                n = mybir.InstNoOp(name=f"wsplit-{type(self)._wsplit_n}")
                n.engine = inst.engine
                n.sync_info = mybir.SyncInfo(on_wait=[w], on_update=[])
                self._add_instruction(n)
        return super()._commit_instruction(inst, lazy_reg_writes)

    def _drain_and_barrier(self, tick_clock, wait_clock):
        from concourse.vector_clock import ScopedClock

        nc = self.nc
        collector = nc.sync.nop(nofuse=True)
        wait_clock.add_sem_waits(
            collector.ins, ScopedClock({None: tick_clock.global_clock})
        )
        waits = list(collector.ins.sync_info.on_wait)
        collector.ins.sync_info = mybir.SyncInfo(on_wait=[], on_update=[])
        for w in waits:
            n = nc.sync.nop(nofuse=True)
            n.ins.sync_info = mybir.SyncInfo(on_wait=[w], on_update=[])
        nc.sync.drain()
        nc.all_engine_barrier()
        assert self.sems is not None
        popped = nc._tile_sem_poison_stack.pop()
        assert popped is self._sem_poison
        nc.clear_and_free_semaphores(list(self.sems.allocated().values()))
        nc.all_engine_barrier()


AF = mybir.ActivationFunctionType


def build_nc(fm: bool = True, has_bias: bool = False):
    nc = bass.Bass(target_bir_lowering=False)

    xr = nc.declare_dram_parameter("xr", [T, H], bf16, isOutput=False).ap()
    xt = nc.declare_dram_parameter("xt", [H, T], bf16, isOutput=False).ap()
    wq = nc.declare_dram_parameter("wq", [H, FPC], bf16, isOutput=False).ap()
    wk = nc.declare_dram_parameter("wk", [H, FPC], bf16, isOutput=False).ap()
    wv = nc.declare_dram_parameter("wv", [H, FPC], bf16, isOutput=False).ap()
    # negated column sums of the (bf16-rounded) weights, and bias rows
    ncsq = nc.declare_dram_parameter("ncsq", [1, FPC], f32r, isOutput=False).ap()
    ncsk = nc.declare_dram_parameter("ncsk", [1, FPC], f32r, isOutput=False).ap()
    ncsv = nc.declare_dram_parameter("ncsv", [1, FPC], f32r, isOutput=False).ap()
    if has_bias:
        bqr = nc.declare_dram_parameter("bqr", [1, FPC], f32r, isOutput=False).ap()
        bkr = nc.declare_dram_parameter("bkr", [1, FPC], f32r, isOutput=False).ap()
        bvr = nc.declare_dram_parameter("bvr", [1, FPC], f32r, isOutput=False).ap()
    abias = nc.declare_dram_parameter(
        "abias", [128, B * HPC, S // 128], f32, isOutput=False
    ).ap()
    if fm:
        # multiplicative key weights exp(abias + NEG) for the narrow pass-2
        m2 = nc.declare_dram_parameter(
            "m2", [128, B * HPC, NP2], f32, isOutput=False
        ).ap()
    wout = nc.declare_dram_parameter("wout", [FPC, H], bf16, isOutput=False).ap()
    out = nc.declare_dram_parameter("out", [T, H], f32, isOutput=True).ap()

    # DRAM scratch
    qT_s = nc.dram_tensor("qT_s", [HPC, 128, T], bf16).ap()
    kT_s = nc.dram_tensor("kT_s", [HPC, 128, T], bf16).ap()
    v_s = nc.dram_tensor("v_s", [T, FPC], bf16).ap()

    xt_r = xt.rearrange("(kt p) t -> p kt t", p=128)
    w_r = {
        "q": wq.rearrange("(kt p) f -> p kt f", p=128),
        "k": wk.rearrange("(kt p) f -> p kt f", p=128),
        "v": wv.rearrange("(kt p) f -> p kt f", p=128),
    }

    with PatchedTileContext(nc) as tc:
        with tc.tile_pool(name="singles", bufs=1) as singles:
            identity = singles.tile([128, 128], f32)
            masks.make_identity(nc, identity[:])
            ones_f = singles.tile([128, 128], f32)
            nc.vector.memset(ones_f[:], 1.0)
            ones_r = singles.tile([128, 128], f32r)
            nc.scalar.activation(out=ones_r[:], in_=ones_f[:], func=AF.Copy)
            ones_bf = singles.tile([128, 1], bf16)
            with nc.allow_low_precision(reason="exact ones"):
                nc.vector.tensor_copy(out=ones_bf[:], in_=ones_f[:, 0:1])
            eps_t = singles.tile([128, 1], f32)
            nc.vector.memset(eps_t[:], LN_EPS)
            # additive causal tiles, one per diagonal offset d = (k0-q0)/128
            causal = singles.tile([128, 4, QTILE], f32)
            nc.gpsimd.memset(causal[:], 0.0)
            for d in range(4):
                nc.gpsimd.affine_select(
                    out=causal[:, d, :],
                    in_=causal[:, d, :],
                    compare_op=mybir.AluOpType.is_ge,
                    fill=NEG,
                    base=-(128 * d),
                    pattern=[[1, QTILE]],
                    channel_multiplier=-1,
                )
            # row constants (csum / bias rows)
            ncs_c = {}
            for nm, src in (("q", ncsq), ("k", ncsk), ("v", ncsv)):
                t = singles.tile([1, FPC], f32r, name=f"ncs_{nm}")
                nc.gpsimd.dma_start(out=t[:], in_=src)
                ncs_c[nm] = t
            b_c = {}
            if has_bias:
                for nm, bsrc in (("q", bqr), ("k", bkr), ("v", bvr)):
                    t = singles.tile([65, FPC], f32r, name=f"b_{nm}")
                    nc.gpsimd.dma_start(out=t[64:65, :], in_=bsrc)
                    b_c[nm] = t
            ab_c = singles.tile([128, B * HPC, S // 128], f32)
            nc.gpsimd.dma_start(out=ab_c[:], in_=abias)
            if fm:
                m2_c = singles.tile([128, B * HPC, NP2], f32)
                nc.gpsimd.dma_start(out=m2_c[:], in_=m2)
                m2_cb = singles.tile([128, B * HPC, NP2], bf16)
                nc.scalar.activation(out=m2_cb[:], in_=m2_c[:], func=AF.Copy)

            # ---------------- Phase A: LN stats + QKV gemms ----------------
            with tc.tile_pool(name="xrp", bufs=4) as xrp, \
                 tc.tile_pool(name="statp", bufs=4) as statp, \
                 tc.tile_pool(name="mvp", bufs=8) as mvp, \
                 tc.tile_pool(name="srowp", bufs=2) as srowp, \
                 tc.tile_pool(name="rssp", bufs=2) as rssp, \
                 tc.tile_pool(name="xtp", bufs=2) as xtp, \
                 tc.tile_pool(name="wp", bufs=2) as wp, \
                 tc.tile_pool(name="stp", bufs=6) as stp, \
                 tc.tile_pool(name="strp", bufs=1, space="PSUM") as strp, \
                 tc.tile_pool(name="rsbp", bufs=2, space="PSUM") as rsbp, \
                 tc.tile_pool(name="qpp", bufs=5, space="PSUM") as qpp:

                def emit_stats(c):
                    """LN stats for chunk c: returns (srow, rsb, mv tiles).
                    srow [65, CHUNK] f32r rows at PE-legal base partitions:
                    0=mean, 32=rstd, 64=std.
                    rsb = rstd broadcast [128, CHUNK] (PSUM f32)."""
                    statrow = strp.tile([65, CHUNK], f32, tag="statrow")
                    mvs = []
                    for tt in range(CHUNK // 128):
                        g = c * (CHUNK // 128) + tt
                        xr_t = xrp.tile([128, H], bf16)
                        nc.sync.dma_start(
                            out=xr_t[:], in_=xr[g * 128:(g + 1) * 128, :]
                        )
                        stats = statp.tile([128, H // 512, 6], f32)
                        xg = xr_t[:].rearrange("p (n f) -> p n f", f=512)
                        for n in range(H // 512):
                            nc.vector.bn_stats(out=stats[:, n, :], in_=xg[:, n, :])
                        mv = mvp.tile([128, 3], f32)
                        nc.vector.bn_aggr(out=mv[:, 0:2], in_=stats[:])
                        nc.scalar.activation(
                            out=mv[:, 2:3], in_=mv[:, 1:2], func=AF.Sqrt,
                            bias=eps_t[:], scale=1.0,
                        )
                        nc.vector.reciprocal(out=mv[:, 1:2], in_=mv[:, 2:3])
                        for i in range(3):
                            nc.tensor.matmul(
                                statrow[
                                    32 * i:32 * i + 1,
                                    tt * 128:(tt + 1) * 128,
                                ],
                                lhsT=mv[:, i:i + 1], rhs=identity[:],
                                start=True, stop=True,
                            )
                        mvs.append(mv)
                    srow = srowp.tile([65, CHUNK], f32r, tag="srow")
                    with nc.allow_low_precision(reason="f32r matmul operand"):
                        nc.vector.tensor_copy(out=srow[:], in_=statrow[:])
                    rsb = rsbp.tile([128, CHUNK], f32, tag="rsb")
                    nc.tensor.matmul(
                        rsb[:], lhsT=ones_r[32:33, :], rhs=srow[32:33, :],
                        start=True, stop=True,
                    )
                    rsbs = rssp.tile([128, CHUNK], f32, tag="rsbs")
                    nc.scalar.activation(out=rsbs[:], in_=rsb[:], func=AF.Copy)
                    return srow, rsbs, mvs

                def load_slab(nm):
                    ws = wp.tile([128, KT, FPC], bf16, tag="wslab")
                    nc.sync.dma_start(out=ws[:], in_=w_r[nm])
                    return ws

                def gemm_block(nm, ws, xt_t, srow, rsb, mvs, c0):
                    flip = nm in ("q", "k")
                    pss = [
                        qpp.tile([128, CHUNK], f32, tag="pss", name=f"pss{f}")
                        for f in range(4)
                    ]
                    for f in range(4):
                        fs = slice(f * 128, (f + 1) * 128)
                        if flip:
                            # psum[d, tok] -= csum[d] * mu[tok]
                            nc.tensor.matmul(
                                pss[f][:], lhsT=ncs_c[nm][0:1, fs],
                                rhs=srow[0:1, :], start=True, stop=False,
                            )
                            if has_bias:
                                nc.tensor.matmul(
                                    pss[f][:], lhsT=b_c[nm][64:65, fs],
                                    rhs=srow[64:65, :], start=False, stop=False,
                                )
                        else:
                            # psum[tok, d] -= mu[tok] * csum[d]
                            nc.tensor.matmul(
                                pss[f][:], lhsT=srow[0:1, fs],
                                rhs=ncs_c[nm][0:1, :], start=True, stop=False,
                            )
                            if has_bias:
                                nc.tensor.matmul(
                                    pss[f][:], lhsT=srow[64:65, fs],
                                    rhs=b_c[nm][64:65, :], start=False, stop=False,
                                )
                    for kt in range(KT):
                        for f in range(4):
                            fs = slice(f * 128, (f + 1) * 128)
                            if flip:
                                nc.tensor.matmul(
                                    pss[f][:], lhsT=ws[:, kt, fs],
                                    rhs=xt_t[:, kt, :],
                                    start=False, stop=(kt == KT - 1),
                                )
                            else:
                                nc.tensor.matmul(
                                    pss[f][:], lhsT=xt_t[:, kt, fs],
                                    rhs=ws[:, kt, :],
                                    start=False, stop=(kt == KT - 1),
                                )
                    dst = {"q": qT_s, "k": kT_s}.get(nm)
                    for f in range(4):
                        st = stp.tile([128, CHUNK], bf16, tag="st", name=f"st{f}")
                        with nc.allow_low_precision(reason="bf16 scratch"):
                            if flip:
                                nc.vector.tensor_mul(
                                    out=st[:], in0=pss[f][:], in1=rsb[:]
                                )
                                nc.sync.dma_start(
                                    out=dst[f, :, c0:c0 + CHUNK], in_=st[:]
                                )
                            else:
                                nc.vector.tensor_scalar_mul(
                                    out=st[:], in0=pss[f][:],
                                    scalar1=mvs[f][:, 1:2],
                                )
                                nc.sync.dma_start(
                                    out=v_s[c0 + f * 128:c0 + (f + 1) * 128, :],
                                    in_=st[:],
                                )

                # prologue: chunk 0 inputs
                xt_ts = {}
                xt_ts[0] = xtp.tile(
                    [128, KT, CHUNK], bf16, tag="xts", name="xts0"
                )
                nc.sync.dma_start(out=xt_ts[0][:], in_=xt_r[:, :, 0:CHUNK])
                slab = {"q": load_slab("q")}
                stat = {0: emit_stats(0)}

                for c in range(NCHUNK):
                    c0 = c * CHUNK
                    if c + 1 < NCHUNK:
                        xt_ts[c + 1] = xtp.tile(
                            [128, KT, CHUNK], bf16, tag="xts",
                            name=f"xts{c + 1}",
                        )
                        nc.sync.dma_start(
                            out=xt_ts[c + 1][:],
                            in_=xt_r[:, :, c0 + CHUNK:c0 + 2 * CHUNK],
                        )
                    slab["k"] = load_slab("k")
                    if c + 1 < NCHUNK:
                        stat[c + 1] = emit_stats(c + 1)
                    srow, rsb, mvs = stat.pop(c)
                    xt_t = xt_ts.pop(c)
                    gemm_block("q", slab.pop("q"), xt_t, srow, rsb, mvs, c0)
                    slab["v"] = load_slab("v")
                    gemm_block("k", slab.pop("k"), xt_t, srow, rsb, mvs, c0)
                    if c + 1 < NCHUNK:
                        slab["q"] = load_slab("q")
                    gemm_block("v", slab.pop("v"), xt_t, srow, rsb, mvs, c0)

            # ------------- Phase B: attention -------------
            with tc.tile_pool(name="ctxp", bufs=1) as ctxp:
                ctx_t = [
                    ctxp.tile([128, S], bf16, tag=f"ctx{u}", name=f"ctx{u}")
                    for u in range(B * HPC)
                ]
                with tc.tile_pool(name="qtp", bufs=2) as qtp, \
                     tc.tile_pool(name="ktp", bufs=2) as ktp, \
                     tc.tile_pool(name="vp", bufs=2) as vp, \
                     tc.tile_pool(name="ep", bufs=6) as ep, \
                     tc.tile_pool(name="esp", bufs=6) as esp, \
                     tc.tile_pool(name="enp", bufs=2) as enp, \
                     tc.tile_pool(name="vnp", bufs=4) as vnp, \
                     tc.tile_pool(name="rp", bufs=3) as rp, \
                     tc.tile_pool(name="scp", bufs=2, space="PSUM") as scp, \
                     tc.tile_pool(name="cpp", bufs=2, space="PSUM") as cpp, \
                     tc.tile_pool(name="smp", bufs=1, space="PSUM") as smp, \
                     tc.tile_pool(name="mscp", bufs=1, space="PSUM") as mscp:
                    for u in range(B * HPC):
                        b, hh = divmod(u, HPC)
                        qt = qtp.tile([128, S], bf16)
                        nc.sync.dma_start(
                            out=qt[:], in_=qT_s[hh, :, b * S:(b + 1) * S]
                        )
                        kt_h = ktp.tile([128, S], bf16)
                        nc.sync.dma_start(
                            out=kt_h[:], in_=kT_s[hh, :, b * S:(b + 1) * S]
                        )
                        vt = vp.tile([128, S // 128, 128], bf16)
                        nc.sync.dma_start(
                            out=vt[:],
                            in_=v_s[
                                b * S:(b + 1) * S, hh * 128:(hh + 1) * 128
                            ].rearrange("(kj p) d -> p kj d", p=128),
                        )
                        for qi in range(S // QTILE):
                            q0 = qi * QTILE
                            nkj = (q0 + QTILE) // 128
                            ctx_ps = cpp.tile([128, QTILE], f32, tag="ctxps")
                            sums = smp.tile([1, QTILE], f32, tag="sums")
                            ext = fm and qi == 0
                            for kjp in range(nkj // 2):
                                sc2 = scp.tile([128, 2, QTILE], f32, tag="sc2")
                                e2 = ep.tile([128, 2, QTILE], bf16, tag="e2")
                                for j in range(2):
                                    kj = 2 * kjp + j
                                    nc.tensor.matmul(
                                        sc2[:, j, :],
                                        lhsT=kt_h[:, kj * 128:(kj + 1) * 128],
                                        rhs=qt[:, q0:q0 + QTILE],
                                        start=True, stop=True,
                                    )
                                    d = kj - (q0 // 128)
                                    if d >= 0:
                                        w = 128 * (d + 1)
                                        nc.vector.tensor_add(
                                            out=sc2[:, j, 0:w],
                                            in0=sc2[:, j, 0:w],
                                            in1=causal[:, d, 0:w],
                                        )
                                    nc.scalar.activation(
                                        out=e2[:, j, :], in_=sc2[:, j, :],
                                        func=AF.Exp,
                                        bias=ab_c[:, u, kj:kj + 1], scale=1.0,
                                    )
                                es = esp.tile([128, QTILE], bf16, tag="es")
                                with nc.allow_low_precision(reason="e pair add"):
                                    nc.vector.tensor_add(
                                        out=es[:], in0=e2[:, 0, :], in1=e2[:, 1, :]
                                    )
                                last = (kjp == nkj // 2 - 1) and not ext
                                nc.tensor.matmul(
                                    sums[:], lhsT=ones_bf[:], rhs=es[:],
                                    start=(kjp == 0), stop=last,
                                )
                                nc.tensor.matmul(
                                    ctx_ps[:], lhsT=vt[:, 2 * kjp, :],
                                    rhs=e2[:, 0, :],
                                    start=(kjp == 0), stop=False,
                                )
                                nc.tensor.matmul(
                                    ctx_ps[:], lhsT=vt[:, 2 * kjp + 1, :],
                                    rhs=e2[:, 1, :],
                                    start=False, stop=last,
                                )
                            if ext:
                                # narrow pass-2: queries 0..127 over kj 4..15
                                # with multiplicative key weights m2
                                for g in range(NP2 // 4):
                                    scn = mscp.tile([128, 4, 128], f32, tag="msc", name="scn")
                                    en = enp.tile([128, 4, 128], bf16, tag="en")
                                    for j in range(4):
                                        kj = 4 + g * 4 + j
                                        nc.tensor.matmul(
                                            scn[:, j, :],
                                            lhsT=kt_h[:, kj * 128:(kj + 1) * 128],
                                            rhs=qt[:, 0:128],
                                            start=True, stop=True,
                                        )
                                    nc.scalar.activation(
                                        out=en[:], in_=scn[:], func=AF.Exp,
                                        scale=1.0,
                                    )
                                    for j in range(4):
                                        kj = 4 + g * 4 + j
                                        ki = g * 4 + j
                                        vtn = vnp.tile([128, 128], bf16, tag="vtn")
                                        with nc.allow_low_precision(
                                            reason="bf16 v scale"
                                        ):
                                            nc.vector.tensor_scalar_mul(
                                                out=vtn[:], in0=vt[:, kj, :],
                                                scalar1=m2_c[:, u, ki:ki + 1],
                                            )
                                        last2 = ki == NP2 - 1
                                        nc.tensor.matmul(
                                            sums[0:1, 0:128],
                                            lhsT=m2_cb[:, u, ki:ki + 1],
                                            rhs=en[:, j, :],
                                            start=False, stop=last2,
                                            skip_group_check=True,
                                        )
                                        nc.tensor.matmul(
                                            ctx_ps[:, 0:128], lhsT=vtn[:],
                                            rhs=en[:, j, :],
                                            start=False, stop=last2,
                                            skip_group_check=True,
                                        )
                            rcp = rp.tile([1, QTILE], f32r, tag="rcp")
                            with nc.allow_low_precision(
                                reason="f32r matmul operand"
                            ):
                                nc.vector.reciprocal(out=rcp[:], in_=sums[:])
                            rsb2 = mscp.tile([128, QTILE], f32, tag="msc", name="rsb2")
                            nc.tensor.matmul(
                                rsb2[:], lhsT=ones_r[0:1, :], rhs=rcp[:],
                                start=True, stop=True,
                            )
                            rsbs2 = rp.tile([128, QTILE], f32, tag="rsbs2")
                            nc.gpsimd.tensor_copy(out=rsbs2[:], in_=rsb2[:])
                            with nc.allow_low_precision(reason="bf16 ctx"):
                                nc.vector.tensor_mul(
                                    out=ctx_t[u][:, q0:q0 + QTILE],
                                    in0=ctx_ps[:], in1=rsbs2[:],
                                )

                # ------------- Phase C: out-proj -------------
                with tc.tile_pool(name="wop", bufs=2) as wop, \
                     tc.tile_pool(name="osp", bufs=6) as osp, \
                     tc.tile_pool(name="opp", bufs=4, space="PSUM") as opp:
                    for hs in range(H // 512):
                        wo_t = wop.tile([128, HPC, 512], bf16)
                        nc.sync.dma_start(
                            out=wo_t[:],
                            in_=wout[:, hs * 512:(hs + 1) * 512].rearrange(
                                "(f p) h -> p f h", p=128
                            ),
                        )
                        for ti in range(T // 128):
                            bb, tloc = divmod(ti, S // 128)
                            ps = opp.tile([128, 512], f32, tag="ops")
                            for f in range(HPC):
                                nc.tensor.matmul(
                                    ps[:],
                                    lhsT=ctx_t[bb * HPC + f][
                                        :, tloc * 128:(tloc + 1) * 128
                                    ],
                                    rhs=wo_t[:, f, :],
                                    start=(f == 0), stop=(f == HPC - 1),
                                )
                            ost = osp.tile([128, 512], f32, tag="ost")
                            nc.scalar.activation(
                                out=ost[:], in_=ps[:], func=AF.Copy
                            )
                            nc.sync.dma_start(
                                out=out[
                                    ti * 128:(ti + 1) * 128,
                                    hs * 512:(hs + 1) * 512,
                                ],
                                in_=ost[:],
                            )
    return nc


_NC_CACHE = {}
_LAST_CFG = (True, False)


def _get_nc(fm: bool = True, has_bias: bool = False):
    key = (fm, has_bias)
    if key not in _NC_CACHE:
        _NC_CACHE[key] = build_nc(fm, has_bias)
    return _NC_CACHE[key]


def _shard_inputs(x, input_mask, alibi, norm_w, norm_b, w_qkv, b_qkv, w_out, b_out):
    scale = np.float32(1.0 / np.sqrt(np.sqrt(np.float32(HD))))
    xf = np.ascontiguousarray(x.reshape(T, H), dtype=np.float32)
    x_bf = xf.astype(BF)
    xT_bf = np.ascontiguousarray(x_bf.T)
    nw = norm_w.astype(np.float32)
    nb = norm_b.astype(np.float32)
    mask_bias = (1.0 - input_mask.astype(np.float32)) * np.float32(NEG)  # [B, S]

    # fully-masked rows live only among the first lz = leading-masked-key
    # count queries; the narrow pass-2 covers queries 0..127.
    lz = 0
    for b in range(B):
        mb = input_mask[b]
        lz = max(lz, int(np.argmax(mb == 1)) if mb.any() else S)
    assert lz < 128, f"leading masked-key run {lz} >= 128 unsupported"
    fm = lz > 0

    in_maps = []
    for c in range(NCORES):
        sl_q = slice(c * FPC, (c + 1) * FPC)
        sl_k = slice(H + c * FPC, H + (c + 1) * FPC)
        sl_v = slice(2 * H + c * FPC, 2 * H + (c + 1) * FPC)
        wq_c = ((nw[:, None] * w_qkv[:, sl_q]) * scale).astype(BF)
        wk_c = ((nw[:, None] * w_qkv[:, sl_k]) * scale).astype(BF)
        wv_c = (nw[:, None] * w_qkv[:, sl_v]).astype(BF)
        # negated column sums over the *rounded* weights (exact mean fold)
        ncsq = -wq_c.astype(np.float32).sum(0)[None, :]
        ncsk = -wk_c.astype(np.float32).sum(0)[None, :]
        ncsv = -wv_c.astype(np.float32).sum(0)[None, :]
        bq_c = ((b_qkv[sl_q] + nb @ w_qkv[:, sl_q]) * scale)[None, :]
        bk_c = ((b_qkv[sl_k] + nb @ w_qkv[:, sl_k]) * scale)[None, :]
        bv_c = (b_qkv[sl_v] + nb @ w_qkv[:, sl_v])[None, :]
        has_bias = bool(
            np.abs(bq_c).max() > 0 or np.abs(bk_c).max() > 0
            or np.abs(bv_c).max() > 0
        )
        ab = np.empty((B * HPC, S), np.float32)
        for b in range(B):
            for hh in range(HPC):
                ab[b * HPC + hh] = alibi[c * HPC + hh, 0, :] + mask_bias[b]
        ab_t = np.ascontiguousarray(
            ab.reshape(B * HPC, S // 128, 128).transpose(2, 0, 1)
        )
        m2_t = np.exp(ab_t[:, :, 4:16].astype(np.float64) + NEG).astype(np.float32)
        im = {
            "xr": x_bf,
            "xt": xT_bf,
            "wq": np.ascontiguousarray(wq_c),
            "wk": np.ascontiguousarray(wk_c),
            "wv": np.ascontiguousarray(wv_c),
            "ncsq": np.ascontiguousarray(ncsq, np.float32),
            "ncsk": np.ascontiguousarray(ncsk, np.float32),
            "ncsv": np.ascontiguousarray(ncsv, np.float32),
            "abias": ab_t,
            "wout": np.ascontiguousarray(w_out[sl_q, :].astype(BF)),
        }
        if has_bias:
            im["bqr"] = np.ascontiguousarray(bq_c, np.float32)
            im["bkr"] = np.ascontiguousarray(bk_c, np.float32)
            im["bvr"] = np.ascontiguousarray(bv_c, np.float32)
        if fm:
            im["m2"] = np.ascontiguousarray(m2_t)
        in_maps.append(im)
    return in_maps, fm, has_bias


def kernel(x, input_mask, alibi, norm_w, norm_b, w_qkv, b_qkv, w_out, b_out):
    from concourse.bass_utils import run_bass_kernel_spmd

    in_maps, fm, has_bias = _shard_inputs(
        np.asarray(x), np.asarray(input_mask), np.asarray(alibi),
        np.asarray(norm_w), np.asarray(norm_b), np.asarray(w_qkv),
        np.asarray(b_qkv), np.asarray(w_out), np.asarray(b_out),
    )
    global _LAST_CFG
    _LAST_CFG = (fm, has_bias)
    nc = _get_nc(fm, has_bias)
    res = run_bass_kernel_spmd(nc, in_maps, core_ids=list(range(NCORES)))
    acc = res.results[0]["out"].astype(np.float32).copy()
    for c in range(1, NCORES):
        acc += res.results[c]["out"]
    acc += np.asarray(b_out, np.float32)[None, :]
    return acc.reshape(B, S, H)


# revision 14
# speedup vs baseline: 1.0073x; 1.0073x over previous
"""DeepSpeed-style self-attention block on 8 Trainium2 NeuronCores.

Tensor-parallel over heads (4 heads/core), DeepSpeed mp_size=8 style:
  - w_qkv column-sharded [H, 3H/8] (per-core wq/wk/wv [H, 512], bf16)
  - w_out row-sharded   [H/8, H]  -> per-core partial outputs
  - layernorm stats computed on device; partial-sum reduction + b_out on host.

Device kernel structure (per core, identical SPMD program, sharded inputs):
  Phase A: the host passes x twice in bf16 - row-major (for LN stats) and
    pre-transposed xT [H, T] (pure layout change).  Per 512-token chunk:
    DVE bn_stats/bn_aggr give mean/rstd/std columns; a tiny PE matmul
    transposes them to rows.  The QKV gemms run directly on RAW xT
    (bf16, full rate); layernorm is folded in algebraically:
       qT = ((W^T xT) - csum(W) (x) mu + b (x) std) * rstd[t]
    The two rank-1 terms are extra K=1 matmuls accumulated into the same
    PSUM; the per-token rstd scale is one DVE multiply against a
    PE-broadcast rstd row (q/k outputs, free-dim tokens) or a per-partition
    tensor_scalar (v output, partition-dim tokens).  norm_w / the
    1/sqrt(sqrt(hd)) scale are folded into the bf16 weights on the host;
    column sums are taken over the *rounded* bf16 weights so the mean
    correction is exact.  Weights stream as one 4MB slab DMA per
    (weight, chunk).  qT/kT [d, tok] and v [tok, d] go to DRAM scratch in
    bf16.
  Phase B: per (batch, head): scoresT = kT^T @ qT -> [k, q] psum blocks
    (two 512-wide kj blocks per 2-bank psum tile); causal handled as
    additive -50 tiles on the diagonal-straddling blocks only (DVE);
    exp via ACT with per-key (mask+alibi) bias; pairs of e tiles are
    pre-added on DVE so the row-sum ones-matmul runs once per 2 kj;
    PV matmul accumulates ctxT [d, q]; 1/sum broadcast via K=1 matmul,
    applied on the PSUM->SBUF ctx write (DVE).
    Rows that DeepSpeed's tied -10000 constants make attend over the
    *whole* sequence (all their visible keys input-masked) can only live
    among the first <128 queries (host-verified); a narrow pass-2 covers
    queries 0..127 over kj 4..15 with multiplicative key weights
    m2 = exp(alibi + maskbias + NEG), accumulated into the same psum.
    (-50 instead of DeepSpeed's -10000 keeps exp() in fp32 range; softmax
    is shift-invariant and masked weights come out ~1e-19, matching the
    reference's exact 0s well below tolerance.)
  Phase C: out-proj partial: out += ctxT^T @ w_out_shard per token tile.

All big matmuls run in bf16 (full rate); accumulation is f32 PSUM.
The walrus build here allows only ONE semaphore wait per instruction;
PatchedTileContext splits surplus Tile-emitted waits onto NoOps.
"""

import numpy as np
import ml_dtypes

import concourse.bass as bass
import concourse.mybir as mybir
import concourse.tile as tile
from concourse import masks

f32 = mybir.dt.float32
f32r = mybir.dt.float32r
bf16 = mybir.dt.bfloat16
BF = ml_dtypes.bfloat16

B, S, H, NH = 2, 2048, 4096, 32
HD = H // NH            # 128 head dim
NCORES = 8
HPC = NH // NCORES      # 4 heads per core
FPC = HPC * HD          # 512 sharded features per core
T = B * S               # 4096 tokens
KT = H // 128           # 32 contraction tiles
CHUNK = 512             # tokens per QKV chunk
NCHUNK = T // CHUNK     # 8
QTILE = 512             # query block in attention
NP2 = 12                # pass-2 kj blocks (kj 4..15) for fully-masked rows
LN_EPS = 1e-5
NEG = -50.0             # soft mask value (see module docstring)


class PatchedTileContext(tile.TileContext):
    """This container's walrus build rejects >1 sync-wait per instruction;
    split surplus waits onto preceding same-engine NoOps."""

    _wsplit_n = 0

    def _commit_instruction(self, inst, lazy_reg_writes: bool = True):
        si = inst.sync_info
        if si is not None and si.on_wait and len(si.on_wait) > 1:
            waits = list(si.on_wait)
            inst.sync_info = mybir.SyncInfo(
                on_wait=[waits[-1]], on_update=list(si.on_update or [])
            )
            for w in waits[:-1]:
                type(self)._wsplit_n += 1
                n = mybir.InstNoOp(name=f"wsplit-{type(self)._wsplit_n}")
                n.engine = inst.engine
                n.sync_info = mybir.SyncInfo(on_wait=[w], on_update=[])
                self._add_instruction(n)
        return super()._commit_instruction(inst, lazy_reg_writes)

    def _drain_and_barrier(self, tick_clock, wait_clock):
        from concourse.vector_clock import ScopedClock

        nc = self.nc
        collector = nc.sync.nop(nofuse=True)
        wait_clock.add_sem_waits(
            collector.ins, ScopedClock({None: tick_clock.global_clock})
        )
        waits = list(collector.ins.sync_info.on_wait)
        collector.ins.sync_info = mybir.SyncInfo(on_wait=[], on_update=[])
        for w in waits:
            n = nc.sync.nop(nofuse=True)
            n.ins.sync_info = mybir.SyncInfo(on_wait=[w], on_update=[])
        nc.sync.drain()
        nc.all_engine_barrier()
        assert self.sems is not None
        popped = nc._tile_sem_poison_stack.pop()
        assert popped is self._sem_poison
        nc.clear_and_free_semaphores(list(self.sems.allocated().values()))
        nc.all_engine_barrier()


AF = mybir.ActivationFunctionType


def build_nc(fm: bool = True, has_bias: bool = False):
    nc = bass.Bass(target_bir_lowering=False)

    xr = nc.declare_dram_parameter("xr", [T, H], bf16, isOutput=False).ap()
    xt = nc.declare_dram_parameter("xt", [H, T], bf16, isOutput=False).ap()
    wq = nc.declare_dram_parameter("wq", [H, FPC], bf16, isOutput=False).ap()
    wk = nc.declare_dram_parameter("wk", [H, FPC], bf16, isOutput=False).ap()
    wv = nc.declare_dram_parameter("wv", [H, FPC], bf16, isOutput=False).ap()
    # negated column sums of the (bf16-rounded) weights, and bias rows
    ncsq = nc.declare_dram_parameter("ncsq", [1, FPC], f32r, isOutput=False).ap()
    ncsk = nc.declare_dram_parameter("ncsk", [1, FPC], f32r, isOutput=False).ap()
    ncsv = nc.declare_dram_parameter("ncsv", [1, FPC], f32r, isOutput=False).ap()
    if has_bias:
        bqr = nc.declare_dram_parameter("bqr", [1, FPC], f32r, isOutput=False).ap()
        bkr = nc.declare_dram_parameter("bkr", [1, FPC], f32r, isOutput=False).ap()
        bvr = nc.declare_dram_parameter("bvr", [1, FPC], f32r, isOutput=False).ap()
    abias = nc.declare_dram_parameter(
        "abias", [128, B * HPC, S // 128], f32, isOutput=False
    ).ap()
    if fm:
        # multiplicative key weights exp(abias + NEG) for the narrow pass-2
        m2 = nc.declare_dram_parameter(
            "m2", [128, B * HPC, NP2], f32, isOutput=False
        ).ap()
    wout = nc.declare_dram_parameter("wout", [FPC, H], bf16, isOutput=False).ap()
    out = nc.declare_dram_parameter("out", [T, H], f32, isOutput=True).ap()

    # DRAM scratch
    qT_s = nc.dram_tensor("qT_s", [HPC, 128, T], bf16).ap()
    kT_s = nc.dram_tensor("kT_s", [HPC, 128, T], bf16).ap()
    v_s = nc.dram_tensor("v_s", [T, FPC], bf16).ap()

    xt_r = xt.rearrange("(kt p) t -> p kt t", p=128)
    w_r = {
        "q": wq.rearrange("(kt p) f -> p kt f", p=128),
        "k": wk.rearrange("(kt p) f -> p kt f", p=128),
        "v": wv.rearrange("(kt p) f -> p kt f", p=128),
    }

    with PatchedTileContext(nc) as tc:
        with tc.tile_pool(name="singles", bufs=1) as singles:
            identity = singles.tile([128, 128], f32)
            masks.make_identity(nc, identity[:])
            ones_f = singles.tile([128, 128], f32)
            nc.vector.memset(ones_f[:], 1.0)
            ones_r = singles.tile([128, 128], f32r)
            nc.scalar.activation(out=ones_r[:], in_=ones_f[:], func=AF.Copy)
            ones_bf = singles.tile([128, 1], bf16)
            with nc.allow_low_precision(reason="exact ones"):
                nc.vector.tensor_copy(out=ones_bf[:], in_=ones_f[:, 0:1])
            eps_t = singles.tile([128, 1], f32)
            nc.vector.memset(eps_t[:], LN_EPS)
            # additive causal tiles, one per diagonal offset d = (k0-q0)/128
            causal = singles.tile([128, 4, QTILE], f32)
            nc.gpsimd.memset(causal[:], 0.0)
            for d in range(4):
                nc.gpsimd.affine_select(
                    out=causal[:, d, :],
                    in_=causal[:, d, :],
                    compare_op=mybir.AluOpType.is_ge,
                    fill=NEG,
                    base=-(128 * d),
                    pattern=[[1, QTILE]],
                    channel_multiplier=-1,
                )
            # row constants (csum / bias rows)
            ncs_c = {}
            for nm, src in (("q", ncsq), ("k", ncsk), ("v", ncsv)):
                t = singles.tile([1, FPC], f32r, name=f"ncs_{nm}")
                nc.gpsimd.dma_start(out=t[:], in_=src)
                ncs_c[nm] = t
            b_c = {}
            if has_bias:
                for nm, bsrc in (("q", bqr), ("k", bkr), ("v", bvr)):
                    t = singles.tile([65, FPC], f32r, name=f"b_{nm}")
                    nc.gpsimd.dma_start(out=t[64:65, :], in_=bsrc)
                    b_c[nm] = t
            ab_c = singles.tile([128, B * HPC, S // 128], f32)
            nc.gpsimd.dma_start(out=ab_c[:], in_=abias)
            if fm:
                m2_c = singles.tile([128, B * HPC, NP2], f32)
                nc.gpsimd.dma_start(out=m2_c[:], in_=m2)
                m2_cb = singles.tile([128, B * HPC, NP2], bf16)
                nc.scalar.activation(out=m2_cb[:], in_=m2_c[:], func=AF.Copy)

            # ---------------- Phase A: LN stats + QKV gemms ----------------
            with tc.tile_pool(name="xrp", bufs=4) as xrp, \
                 tc.tile_pool(name="statp", bufs=4) as statp, \
                 tc.tile_pool(name="mvp", bufs=8) as mvp, \
                 tc.tile_pool(name="srowp", bufs=2) as srowp, \
                 tc.tile_pool(name="rssp", bufs=2) as rssp, \
                 tc.tile_pool(name="xtp", bufs=2) as xtp, \
                 tc.tile_pool(name="wp", bufs=2) as wp, \
                 tc.tile_pool(name="stp", bufs=6) as stp, \
                 tc.tile_pool(name="strp", bufs=1, space="PSUM") as strp, \
                 tc.tile_pool(name="rsbp", bufs=2, space="PSUM") as rsbp, \
                 tc.tile_pool(name="qpp", bufs=5, space="PSUM") as qpp:

                def emit_stats(c):
                    """LN stats for chunk c: returns (srow, rsb, mv tiles).
                    srow [65, CHUNK] f32r rows at PE-legal base partitions:
                    0=mean, 32=rstd, 64=std.
                    rsb = rstd broadcast [128, CHUNK] (PSUM f32)."""
                    statrow = strp.tile([65, CHUNK], f32, tag="statrow")
                    mvs = []
                    for tt in range(CHUNK // 128):
                        g = c * (CHUNK // 128) + tt
                        xr_t = xrp.tile([128, H], bf16)
                        nc.sync.dma_start(
                            out=xr_t[:], in_=xr[g * 128:(g + 1) * 128, :]
                        )
                        stats = statp.tile([128, H // 512, 6], f32)
                        xg = xr_t[:].rearrange("p (n f) -> p n f", f=512)
                        for n in range(H // 512):
                            nc.vector.bn_stats(out=stats[:, n, :], in_=xg[:, n, :])
                        mv = mvp.tile([128, 3], f32)
                        nc.vector.bn_aggr(out=mv[:, 0:2], in_=stats[:])
                        nc.scalar.activation(
                            out=mv[:, 2:3], in_=mv[:, 1:2], func=AF.Sqrt,
                            bias=eps_t[:], scale=1.0,
                        )
                        nc.vector.reciprocal(out=mv[:, 1:2], in_=mv[:, 2:3])
                        for i in range(3):
                            nc.tensor.matmul(
                                statrow[
                                    32 * i:32 * i + 1,
                                    tt * 128:(tt + 1) * 128,
                                ],
                                lhsT=mv[:, i:i + 1], rhs=identity[:],
                                start=True, stop=True,
                            )
                        mvs.append(mv)
                    srow = srowp.tile([65, CHUNK], f32r, tag="srow")
                    with nc.allow_low_precision(reason="f32r matmul operand"):
                        nc.vector.tensor_copy(out=srow[:], in_=statrow[:])
                    rsb = rsbp.tile([128, CHUNK], f32, tag="rsb")
                    nc.tensor.matmul(
                        rsb[:], lhsT=ones_r[32:33, :], rhs=srow[32:33, :],
                        start=True, stop=True,
                    )
                    rsbs = rssp.tile([128, CHUNK], f32, tag="rsbs")
                    nc.scalar.activation(out=rsbs[:], in_=rsb[:], func=AF.Copy)
                    return srow, rsbs, mvs

                def load_slab(nm):
                    ws = wp.tile([128, KT, FPC], bf16, tag="wslab")
                    nc.sync.dma_start(out=ws[:], in_=w_r[nm])
                    return ws

                def gemm_block(nm, ws, xt_t, srow, rsb, mvs, c0):
                    flip = nm in ("q", "k")
                    pss = [
                        qpp.tile([128, CHUNK], f32, tag="pss", name=f"pss{f}")
                        for f in range(4)
                    ]
                    for f in range(4):
                        fs = slice(f * 128, (f + 1) * 128)
                        if flip:
                            # psum[d, tok] -= csum[d] * mu[tok]
                            nc.tensor.matmul(
                                pss[f][:], lhsT=ncs_c[nm][0:1, fs],
                                rhs=srow[0:1, :], start=True, stop=False,
                            )
                            if has_bias:
                                nc.tensor.matmul(
                                    pss[f][:], lhsT=b_c[nm][64:65, fs],
                                    rhs=srow[64:65, :], start=False, stop=False,
                                )
                        else:
                            # psum[tok, d] -= mu[tok] * csum[d]
                            nc.tensor.matmul(
                                pss[f][:], lhsT=srow[0:1, fs],
                                rhs=ncs_c[nm][0:1, :], start=True, stop=False,
                            )
                            if has_bias:
                                nc.tensor.matmul(
                                    pss[f][:], lhsT=srow[64:65, fs],
                                    rhs=b_c[nm][64:65, :], start=False, stop=False,
                                )
                    for kt in range(KT):
                        for f in range(4):
                            fs = slice(f * 128, (f + 1) * 128)
                            if flip:
                                nc.tensor.matmul(
                                    pss[f][:], lhsT=ws[:, kt, fs],
                                    rhs=xt_t[:, kt, :],
                                    start=False, stop=(kt == KT - 1),
                                )
                            else:
                                nc.tensor.matmul(
                                    pss[f][:], lhsT=xt_t[:, kt, fs],
                                    rhs=ws[:, kt, :],
                                    start=False, stop=(kt == KT - 1),
                                )
                    dst = {"q": qT_s, "k": kT_s}.get(nm)
                    for f in range(4):
                        st = stp.tile([128, CHUNK], bf16, tag="st", name=f"st{f}")
                        with nc.allow_low_precision(reason="bf16 scratch"):
                            if flip:
                                nc.vector.tensor_mul(
                                    out=st[:], in0=pss[f][:], in1=rsb[:]
                                )
                                nc.sync.dma_start(
                                    out=dst[f, :, c0:c0 + CHUNK], in_=st[:]
                                )
                            else:
                                nc.vector.tensor_scalar_mul(
                                    out=st[:], in0=pss[f][:],
                                    scalar1=mvs[f][:, 1:2],
                                )
                                nc.sync.dma_start(
                                    out=v_s[c0 + f * 128:c0 + (f + 1) * 128, :],
                                    in_=st[:],
                                )

                # prologue: chunk 0 inputs
                xt_ts = {}
                xt_ts[0] = xtp.tile(
                    [128, KT, CHUNK], bf16, tag="xts", name="xts0"
                )
                nc.sync.dma_start(out=xt_ts[0][:], in_=xt_r[:, :, 0:CHUNK])
                slab = {"q": load_slab("q")}
                stat = {0: emit_stats(0)}

                for c in range(NCHUNK):
                    c0 = c * CHUNK
                    if c + 1 < NCHUNK:
                        xt_ts[c + 1] = xtp.tile(
                            [128, KT, CHUNK], bf16, tag="xts",
                            name=f"xts{c + 1}",
                        )
                        nc.sync.dma_start(
                            out=xt_ts[c + 1][:],
                            in_=xt_r[:, :, c0 + CHUNK:c0 + 2 * CHUNK],
                        )
                    slab["k"] = load_slab("k")
                    if c + 1 < NCHUNK:
                        stat[c + 1] = emit_stats(c + 1)
                    srow, rsb, mvs = stat.pop(c)
                    xt_t = xt_ts.pop(c)
                    gemm_block("q", slab.pop("q"), xt_t, srow, rsb, mvs, c0)
                    slab["v"] = load_slab("v")
                    gemm_block("k", slab.pop("k"), xt_t, srow, rsb, mvs, c0)
                    if c + 1 < NCHUNK:
                        slab["q"] = load_slab("q")
                    gemm_block("v", slab.pop("v"), xt_t, srow, rsb, mvs, c0)

            # ------------- Phase B: attention -------------
            with tc.tile_pool(name="ctxp", bufs=1) as ctxp:
                ctx_t = [
                    ctxp.tile([128, S], bf16, tag=f"ctx{u}", name=f"ctx{u}")
                    for u in range(B * HPC)
                ]
                with tc.tile_pool(name="qtp", bufs=2) as qtp, \
                     tc.tile_pool(name="ktp", bufs=2) as ktp, \
                     tc.tile_pool(name="vp", bufs=2) as vp, \
                     tc.tile_pool(name="ep", bufs=6) as ep, \
                     tc.tile_pool(name="esp", bufs=6) as esp, \
                     tc.tile_pool(name="enp", bufs=2) as enp, \
                     tc.tile_pool(name="vnp", bufs=4) as vnp, \
                     tc.tile_pool(name="rp", bufs=3) as rp, \
                     tc.tile_pool(name="scp", bufs=2, space="PSUM") as scp, \
                     tc.tile_pool(name="cpp", bufs=2, space="PSUM") as cpp, \
                     tc.tile_pool(name="smp", bufs=1, space="PSUM") as smp, \
                     tc.tile_pool(name="mscp", bufs=1, space="PSUM") as mscp:
                    for u in range(B * HPC):
                        b, hh = divmod(u, HPC)
                        qt = qtp.tile([128, S], bf16)
                        nc.sync.dma_start(
                            out=qt[:], in_=qT_s[hh, :, b * S:(b + 1) * S]
                        )
                        kt_h = ktp.tile([128, S], bf16)
                        nc.sync.dma_start(
                            out=kt_h[:], in_=kT_s[hh, :, b * S:(b + 1) * S]
                        )
                        vt = vp.tile([128, S // 128, 128], bf16)
                        nc.sync.dma_start(
                            out=vt[:],
                            in_=v_s[
                                b * S:(b + 1) * S, hh * 128:(hh + 1) * 128
                            ].rearrange("(kj p) d -> p kj d", p=128),
                        )
                        for qi in range(S // QTILE):
                            q0 = qi * QTILE
                            nkj = (q0 + QTILE) // 128
                            ctx_ps = cpp.tile([128, QTILE], f32, tag="ctxps")
                            sums = smp.tile([1, QTILE], f32, tag="sums")
                            ext = fm and qi == 0
                            for kjp in range(nkj // 2):
                                sc2 = scp.tile([128, 2, QTILE], f32, tag="sc2")
                                e2 = ep.tile([128, 2, QTILE], bf16, tag="e2")
                                for j in range(2):
                                    kj = 2 * kjp + j
                                    nc.tensor.matmul(
                                        sc2[:, j, :],
                                        lhsT=kt_h[:, kj * 128:(kj + 1) * 128],
                                        rhs=qt[:, q0:q0 + QTILE],
                                        start=True, stop=True,
                                    )
                                    d = kj - (q0 // 128)
                                    if d >= 0:
                                        w = 128 * (d + 1)
                                        nc.vector.tensor_add(
                                            out=sc2[:, j, 0:w],
                                            in0=sc2[:, j, 0:w],
                                            in1=causal[:, d, 0:w],
                                        )
                                    nc.scalar.activation(
                                        out=e2[:, j, :], in_=sc2[:, j, :],
                                        func=AF.Exp,
                                        bias=ab_c[:, u, kj:kj + 1], scale=1.0,
                                    )
                                es = esp.tile([128, QTILE], bf16, tag="es")
                                with nc.allow_low_precision(reason="e pair add"):
                                    nc.vector.tensor_add(
                                        out=es[:], in0=e2[:, 0, :], in1=e2[:, 1, :]
                                    )
                                last = (kjp == nkj // 2 - 1) and not ext
                                nc.tensor.matmul(
                                    sums[:], lhsT=ones_bf[:], rhs=es[:],
                                    start=(kjp == 0), stop=last,
                                )
                                nc.tensor.matmul(
                                    ctx_ps[:], lhsT=vt[:, 2 * kjp, :],
                                    rhs=e2[:, 0, :],
                                    start=(kjp == 0), stop=False,
                                )
                                nc.tensor.matmul(
                                    ctx_ps[:], lhsT=vt[:, 2 * kjp + 1, :],
                                    rhs=e2[:, 1, :],
                                    start=False, stop=last,
                                )
                            if ext:
                                # narrow pass-2: queries 0..127 over kj 4..15
                                # with multiplicative key weights m2
                                for g in range(NP2 // 4):
                                    scn = mscp.tile([128, 4, 128], f32, tag="msc", name="scn")
                                    en = enp.tile([128, 4, 128], bf16, tag="en")
                                    for j in range(4):
                                        kj = 4 + g * 4 + j
                                        nc.tensor.matmul(
                                            scn[:, j, :],
                                            lhsT=kt_h[:, kj * 128:(kj + 1) * 128],
                                            rhs=qt[:, 0:128],
                                            start=True, stop=True,
                                        )
                                    nc.scalar.activation(
                                        out=en[:], in_=scn[:], func=AF.Exp,
                                        scale=1.0,
                                    )
                                    for j in range(4):
                                        kj = 4 + g * 4 + j
                                        ki = g * 4 + j
                                        vtn = vnp.tile([128, 128], bf16, tag="vtn")
                                        with nc.allow_low_precision(
                                            reason="bf16 v scale"
                                        ):
                                            nc.vector.tensor_scalar_mul(
                                                out=vtn[:], in0=vt[:, kj, :],
                                                scalar1=m2_c[:, u, ki:ki + 1],
                                            )
                                        last2 = ki == NP2 - 1
                                        nc.tensor.matmul(
                                            sums[0:1, 0:128],
                                            lhsT=m2_cb[:, u, ki:ki + 1],
                                            rhs=en[:, j, :],
                                            start=False, stop=last2,
                                            skip_group_check=True,
                                        )
                                        nc.tensor.matmul(
                                            ctx_ps[:, 0:128], lhsT=vtn[:],
                                            rhs=en[:, j, :],
                                            start=False, stop=last2,
                                            skip_group_check=True,
                                        )
                            rcp = rp.tile([1, QTILE], f32r, tag="rcp")
                            with nc.allow_low_precision(
                                reason="f32r matmul operand"
                            ):
                                nc.vector.reciprocal(out=rcp[:], in_=sums[:])
                            rsb2 = mscp.tile([128, QTILE], f32, tag="msc", name="rsb2")
                            nc.tensor.matmul(
                                rsb2[:], lhsT=ones_r[0:1, :], rhs=rcp[:],
                                start=True, stop=True,
                            )
                            rsbs2 = rp.tile([128, QTILE], f32, tag="rsbs2")
                            nc.vector.tensor_copy(out=rsbs2[:], in_=rsb2[:])
                            with nc.allow_low_precision(reason="bf16 ctx"):
                                nc.vector.tensor_mul(
                                    out=ctx_t[u][:, q0:q0 + QTILE],
                                    in0=ctx_ps[:], in1=rsbs2[:],
                                )

                # ------------- Phase C: out-proj -------------
                with tc.tile_pool(name="wop", bufs=2) as wop, \
                     tc.tile_pool(name="osp", bufs=6) as osp, \
                     tc.tile_pool(name="opp", bufs=4, space="PSUM") as opp:
                    for hs in range(H // 512):
                        wo_t = wop.tile([128, HPC, 512], bf16)
                        nc.sync.dma_start(
                            out=wo_t[:],
                            in_=wout[:, hs * 512:(hs + 1) * 512].rearrange(
                                "(f p) h -> p f h", p=128
                            ),
                        )
                        for ti in range(T // 128):
                            bb, tloc = divmod(ti, S // 128)
                            ps = opp.tile([128, 512], f32, tag="ops")
                            for f in range(HPC):
                                nc.tensor.matmul(
                                    ps[:],
                                    lhsT=ctx_t[bb * HPC + f][
                                        :, tloc * 128:(tloc + 1) * 128
                                    ],
                                    rhs=wo_t[:, f, :],
                                    start=(f == 0), stop=(f == HPC - 1),
                                )
                            ost = osp.tile([128, 512], f32, tag="ost")
                            nc.scalar.activation(
                                out=ost[:], in_=ps[:], func=AF.Copy
                            )
                            nc.sync.dma_start(
                                out=out[
                                    ti * 128:(ti + 1) * 128,
                                    hs * 512:(hs + 1) * 512,
                                ],
                                in_=ost[:],
                            )
    return nc


_NC_CACHE = {}
_LAST_CFG = (True, False)


def _get_nc(fm: bool = True, has_bias: bool = False):
    key = (fm, has_bias)
    if key not in _NC_CACHE:
        _NC_CACHE[key] = build_nc(fm, has_bias)
    return _NC_CACHE[key]


def _shard_inputs(x, input_mask, alibi, norm_w, norm_b, w_qkv, b_qkv, w_out, b_out):
    scale = np.float32(1.0 / np.sqrt(np.sqrt(np.float32(HD))))
    xf = np.ascontiguousarray(x.reshape(T, H), dtype=np.float32)
    x_bf = xf.astype(BF)
    xT_bf = np.ascontiguousarray(x_bf.T)
    nw = norm_w.astype(np.float32)
    nb = norm_b.astype(np.float32)
    mask_bias = (1.0 - input_mask.astype(np.float32)) * np.float32(NEG)  # [B, S]

    # fully-masked rows live only among the first lz = leading-masked-key
    # count queries; the narrow pass-2 covers queries 0..127.
    lz = 0
    for b in range(B):
        mb = input_mask[b]
        lz = max(lz, int(np.argmax(mb == 1)) if mb.any() else S)
    assert lz < 128, f"leading masked-key run {lz} >= 128 unsupported"
    fm = lz > 0

    in_maps = []
    for c in range(NCORES):
        sl_q = slice(c * FPC, (c + 1) * FPC)
        sl_k = slice(H + c * FPC, H + (c + 1) * FPC)
        sl_v = slice(2 * H + c * FPC, 2 * H + (c + 1) * FPC)
        wq_c = ((nw[:, None] * w_qkv[:, sl_q]) * scale).astype(BF)
        wk_c = ((nw[:, None] * w_qkv[:, sl_k]) * scale).astype(BF)
        wv_c = (nw[:, None] * w_qkv[:, sl_v]).astype(BF)
        # negated column sums over the *rounded* weights (exact mean fold)
        ncsq = -wq_c.astype(np.float32).sum(0)[None, :]
        ncsk = -wk_c.astype(np.float32).sum(0)[None, :]
        ncsv = -wv_c.astype(np.float32).sum(0)[None, :]
        bq_c = ((b_qkv[sl_q] + nb @ w_qkv[:, sl_q]) * scale)[None, :]
        bk_c = ((b_qkv[sl_k] + nb @ w_qkv[:, sl_k]) * scale)[None, :]
        bv_c = (b_qkv[sl_v] + nb @ w_qkv[:, sl_v])[None, :]
        has_bias = bool(
            np.abs(bq_c).max() > 0 or np.abs(bk_c).max() > 0
            or np.abs(bv_c).max() > 0
        )
        ab = np.empty((B * HPC, S), np.float32)
        for b in range(B):
            for hh in range(HPC):
                ab[b * HPC + hh] = alibi[c * HPC + hh, 0, :] + mask_bias[b]
        ab_t = np.ascontiguousarray(
            ab.reshape(B * HPC, S // 128, 128).transpose(2, 0, 1)
        )
        m2_t = np.exp(ab_t[:, :, 4:16].astype(np.float64) + NEG).astype(np.float32)
        im = {
            "xr": x_bf,
            "xt": xT_bf,
            "wq": np.ascontiguousarray(wq_c),
            "wk": np.ascontiguousarray(wk_c),
            "wv": np.ascontiguousarray(wv_c),
            "ncsq": np.ascontiguousarray(ncsq, np.float32),
            "ncsk": np.ascontiguousarray(ncsk, np.float32),
            "ncsv": np.ascontiguousarray(ncsv, np.float32),
            "abias": ab_t,
            "wout": np.ascontiguousarray(w_out[sl_q, :].astype(BF)),
        }
        if has_bias:
            im["bqr"] = np.ascontiguousarray(bq_c, np.float32)
            im["bkr"] = np.ascontiguousarray(bk_c, np.float32)
            im["bvr"] = np.ascontiguousarray(bv_c, np.float32)
        if fm:
            im["m2"] = np.ascontiguousarray(m2_t)
        in_maps.append(im)
    return in_maps, fm, has_bias


def kernel(x, input_mask, alibi, norm_w, norm_b, w_qkv, b_qkv, w_out, b_out):
    from concourse.bass_utils import run_bass_kernel_spmd

    in_maps, fm, has_bias = _shard_inputs(
        np.asarray(x), np.asarray(input_mask), np.asarray(alibi),
        np.asarray(norm_w), np.asarray(norm_b), np.asarray(w_qkv),
        np.asarray(b_qkv), np.asarray(w_out), np.asarray(b_out),
    )
    global _LAST_CFG
    _LAST_CFG = (fm, has_bias)
    nc = _get_nc(fm, has_bias)
    res = run_bass_kernel_spmd(nc, in_maps, core_ids=list(range(NCORES)))
    acc = res.results[0]["out"].astype(np.float32).copy()
    for c in range(1, NCORES):
        acc += res.results[c]["out"]
    acc += np.asarray(b_out, np.float32)[None, :]
    return acc.reshape(B, S, H)


# revision 15
# speedup vs baseline: 1.0576x; 1.0499x over previous
"""DeepSpeed-style self-attention block on 8 Trainium2 NeuronCores.

Tensor-parallel over heads (4 heads/core), DeepSpeed mp_size=8 style:
  - w_qkv column-sharded [H, 3H/8] (per-core wq/wk/wv [H, 512], bf16)
  - w_out row-sharded   [H/8, H]  -> per-core partial outputs
  - layernorm stats computed on device; partial-sum reduction + b_out on host.

Device kernel structure (per core, identical SPMD program, sharded inputs):
  Phase A: the host passes x twice in bf16 - row-major (for LN stats) and
    pre-transposed xT [H, T] (pure layout change).  Per 512-token chunk:
    DVE bn_stats/bn_aggr give mean/rstd/std columns; a tiny PE matmul
    transposes them to rows.  The QKV gemms run directly on RAW xT
    (bf16, full rate); layernorm is folded in algebraically:
       qT = ((W^T xT) - csum(W) (x) mu + b (x) std) * rstd[t]
    The two rank-1 terms are extra K=1 matmuls accumulated into the same
    PSUM; the per-token rstd scale is one DVE multiply against a
    PE-broadcast rstd row (q/k outputs, free-dim tokens) or a per-partition
    tensor_scalar (v output, partition-dim tokens).  norm_w / the
    1/sqrt(sqrt(hd)) scale are folded into the bf16 weights on the host;
    column sums are taken over the *rounded* bf16 weights so the mean
    correction is exact.  Weights stream as one 4MB slab DMA per
    (weight, chunk).  qT/kT [d, tok] and v [tok, d] go to DRAM scratch in
    bf16.
  Phase B: per (batch, head): scoresT = kT^T @ qT -> [k, q] psum blocks
    (two 512-wide kj blocks per 2-bank psum tile); causal handled as
    additive -50 tiles on the diagonal-straddling blocks only (DVE);
    exp via ACT with per-key (mask+alibi) bias; pairs of e tiles are
    pre-added on DVE so the row-sum ones-matmul runs once per 2 kj;
    PV matmul accumulates ctxT [d, q]; 1/sum broadcast via K=1 matmul,
    applied on the PSUM->SBUF ctx write (DVE).
    Rows that DeepSpeed's tied -10000 constants make attend over the
    *whole* sequence (all their visible keys input-masked) can only live
    among the first <128 queries (host-verified); a narrow pass-2 covers
    queries 0..127 over kj 4..15 with multiplicative key weights
    m2 = exp(alibi + maskbias + NEG), accumulated into the same psum.
    (-50 instead of DeepSpeed's -10000 keeps exp() in fp32 range; softmax
    is shift-invariant and masked weights come out ~1e-19, matching the
    reference's exact 0s well below tolerance.)
  Phase C: out-proj partial: out += ctxT^T @ w_out_shard per token tile.

All big matmuls run in bf16 (full rate); accumulation is f32 PSUM.
The walrus build here allows only ONE semaphore wait per instruction;
PatchedTileContext splits surplus Tile-emitted waits onto NoOps.
"""

import numpy as np
import ml_dtypes

import concourse.bass as bass
import concourse.mybir as mybir
import concourse.tile as tile
from concourse import masks

f32 = mybir.dt.float32
f32r = mybir.dt.float32r
bf16 = mybir.dt.bfloat16
BF = ml_dtypes.bfloat16

B, S, H, NH = 2, 2048, 4096, 32
HD = H // NH            # 128 head dim
NCORES = 8
HPC = NH // NCORES      # 4 heads per core
FPC = HPC * HD          # 512 sharded features per core
T = B * S               # 4096 tokens
KT = H // 128           # 32 contraction tiles
CHUNK = 512             # tokens per QKV chunk
NCHUNK = T // CHUNK     # 8
QTILE = 512             # query block in attention
NP2 = 12                # pass-2 kj blocks (kj 4..15) for fully-masked rows
LN_EPS = 1e-5
NEG = -50.0             # soft mask value (see module docstring)


class PatchedTileContext(tile.TileContext):
    """This container's walrus build rejects >1 sync-wait per instruction;
    split surplus waits onto preceding same-engine NoOps."""

    _wsplit_n = 0

    def _commit_instruction(self, inst, lazy_reg_writes: bool = True):
        si = inst.sync_info
        if si is not None and si.on_wait and len(si.on_wait) > 1:
            waits = list(si.on_wait)
            inst.sync_info = mybir.SyncInfo(
                on_wait=[waits[-1]], on_update=list(si.on_update or [])
            )
            for w in waits[:-1]:
                type(self)._wsplit_n += 1
                n = mybir.InstNoOp(name=f"wsplit-{type(self)._wsplit_n}")
                n.engine = inst.engine
                n.sync_info = mybir.SyncInfo(on_wait=[w], on_update=[])
                self._add_instruction(n)
        return super()._commit_instruction(inst, lazy_reg_writes)

    def _drain_and_barrier(self, tick_clock, wait_clock):
        from concourse.vector_clock import ScopedClock

        nc = self.nc
        collector = nc.sync.nop(nofuse=True)
        wait_clock.add_sem_waits(
            collector.ins, ScopedClock({None: tick_clock.global_clock})
        )
        waits = list(collector.ins.sync_info.on_wait)
        collector.ins.sync_info = mybir.SyncInfo(on_wait=[], on_update=[])
        for w in waits:
            n = nc.sync.nop(nofuse=True)
            n.ins.sync_info = mybir.SyncInfo(on_wait=[w], on_update=[])
        nc.sync.drain()
        nc.all_engine_barrier()
        assert self.sems is not None
        popped = nc._tile_sem_poison_stack.pop()
        assert popped is self._sem_poison
        nc.clear_and_free_semaphores(list(self.sems.allocated().values()))
        nc.all_engine_barrier()


AF = mybir.ActivationFunctionType


def build_nc(fm: bool = True, has_bias: bool = False):
    nc = bass.Bass(target_bir_lowering=False)

    xr = nc.declare_dram_parameter("xr", [T, H], bf16, isOutput=False).ap()
    xt = nc.declare_dram_parameter("xt", [H, T], bf16, isOutput=False).ap()
    wq = nc.declare_dram_parameter("wq", [H, FPC], bf16, isOutput=False).ap()
    wk = nc.declare_dram_parameter("wk", [H, FPC], bf16, isOutput=False).ap()
    wv = nc.declare_dram_parameter("wv", [H, FPC], bf16, isOutput=False).ap()
    # negated column sums of the (bf16-rounded) weights, and bias rows
    ncsq = nc.declare_dram_parameter("ncsq", [1, FPC], f32r, isOutput=False).ap()
    ncsk = nc.declare_dram_parameter("ncsk", [1, FPC], f32r, isOutput=False).ap()
    ncsv = nc.declare_dram_parameter("ncsv", [1, FPC], f32r, isOutput=False).ap()
    if has_bias:
        bqr = nc.declare_dram_parameter("bqr", [1, FPC], f32r, isOutput=False).ap()
        bkr = nc.declare_dram_parameter("bkr", [1, FPC], f32r, isOutput=False).ap()
        bvr = nc.declare_dram_parameter("bvr", [1, FPC], f32r, isOutput=False).ap()
    abias = nc.declare_dram_parameter(
        "abias", [128, B * HPC, S // 128], f32, isOutput=False
    ).ap()
    if fm:
        # multiplicative key weights exp(abias + NEG) for the narrow pass-2
        m2 = nc.declare_dram_parameter(
            "m2", [128, B * HPC, NP2], f32, isOutput=False
        ).ap()
    wout = nc.declare_dram_parameter("wout", [FPC, H], bf16, isOutput=False).ap()
    out = nc.declare_dram_parameter("out", [T, H], f32, isOutput=True).ap()

    # DRAM scratch
    qT_s = nc.dram_tensor("qT_s", [HPC, 128, T], bf16).ap()
    kT_s = nc.dram_tensor("kT_s", [HPC, 128, T], bf16).ap()
    v_s = nc.dram_tensor("v_s", [T, FPC], bf16).ap()

    xt_r = xt.rearrange("(kt p) t -> p kt t", p=128)
    w_r = {
        "q": wq.rearrange("(kt p) f -> p kt f", p=128),
        "k": wk.rearrange("(kt p) f -> p kt f", p=128),
        "v": wv.rearrange("(kt p) f -> p kt f", p=128),
    }

    with PatchedTileContext(nc) as tc:
        with tc.tile_pool(name="singles", bufs=1) as singles:
            identity = singles.tile([128, 128], f32)
            masks.make_identity(nc, identity[:])
            ones_f = singles.tile([128, 128], f32)
            nc.vector.memset(ones_f[:], 1.0)
            ones_r = singles.tile([128, 128], f32r)
            nc.scalar.activation(out=ones_r[:], in_=ones_f[:], func=AF.Copy)
            ones_bf = singles.tile([128, 1], bf16)
            with nc.allow_low_precision(reason="exact ones"):
                nc.vector.tensor_copy(out=ones_bf[:], in_=ones_f[:, 0:1])
            eps_t = singles.tile([128, 1], f32)
            nc.vector.memset(eps_t[:], LN_EPS)
            # additive causal tiles, one per diagonal offset d = (k0-q0)/128
            causal = singles.tile([128, 4, QTILE], f32)
            nc.gpsimd.memset(causal[:], 0.0)
            for d in range(4):
                nc.gpsimd.affine_select(
                    out=causal[:, d, :],
                    in_=causal[:, d, :],
                    compare_op=mybir.AluOpType.is_ge,
                    fill=NEG,
                    base=-(128 * d),
                    pattern=[[1, QTILE]],
                    channel_multiplier=-1,
                )
            # row constants (csum / bias rows)
            ncs_c = {}
            for nm, src in (("q", ncsq), ("k", ncsk), ("v", ncsv)):
                t = singles.tile([1, FPC], f32r, name=f"ncs_{nm}")
                nc.gpsimd.dma_start(out=t[:], in_=src)
                ncs_c[nm] = t
            b_c = {}
            if has_bias:
                for nm, bsrc in (("q", bqr), ("k", bkr), ("v", bvr)):
                    t = singles.tile([65, FPC], f32r, name=f"b_{nm}")
                    nc.gpsimd.dma_start(out=t[64:65, :], in_=bsrc)
                    b_c[nm] = t
            ab_c = singles.tile([128, B * HPC, S // 128], f32)
            nc.gpsimd.dma_start(out=ab_c[:], in_=abias)
            if fm:
                m2_c = singles.tile([128, B * HPC, NP2], f32)
                nc.gpsimd.dma_start(out=m2_c[:], in_=m2)
                m2_cb = singles.tile([128, B * HPC, NP2], bf16)
                nc.scalar.activation(out=m2_cb[:], in_=m2_c[:], func=AF.Copy)

            # ---------------- Phase A: LN stats + QKV gemms ----------------
            with tc.tile_pool(name="xrp", bufs=4) as xrp, \
                 tc.tile_pool(name="statp", bufs=4) as statp, \
                 tc.tile_pool(name="mvp", bufs=8) as mvp, \
                 tc.tile_pool(name="srowp", bufs=2) as srowp, \
                 tc.tile_pool(name="rssp", bufs=2) as rssp, \
                 tc.tile_pool(name="xtp", bufs=2) as xtp, \
                 tc.tile_pool(name="wp", bufs=2) as wp, \
                 tc.tile_pool(name="stp", bufs=6) as stp, \
                 tc.tile_pool(name="strp", bufs=1, space="PSUM") as strp, \
                 tc.tile_pool(name="rsbp", bufs=2, space="PSUM") as rsbp, \
                 tc.tile_pool(name="qpp", bufs=5, space="PSUM") as qpp:

                def emit_stats(c):
                    """LN stats for chunk c: returns (srow, rsb, mv tiles).
                    srow [65, CHUNK] f32r rows at PE-legal base partitions:
                    0=mean, 32=rstd, 64=std.
                    rsb = rstd broadcast [128, CHUNK] (PSUM f32)."""
                    statrow = strp.tile([65, CHUNK], f32, tag="statrow")
                    mvs = []
                    for tt in range(CHUNK // 128):
                        g = c * (CHUNK // 128) + tt
                        xr_t = xrp.tile([128, H], bf16)
                        nc.sync.dma_start(
                            out=xr_t[:], in_=xr[g * 128:(g + 1) * 128, :]
                        )
                        stats = statp.tile([128, H // 512, 6], f32)
                        xg = xr_t[:].rearrange("p (n f) -> p n f", f=512)
                        for n in range(H // 512):
                            nc.vector.bn_stats(out=stats[:, n, :], in_=xg[:, n, :])
                        mv = mvp.tile([128, 3], f32)
                        nc.vector.bn_aggr(out=mv[:, 0:2], in_=stats[:])
                        nc.scalar.activation(
                            out=mv[:, 2:3], in_=mv[:, 1:2], func=AF.Sqrt,
                            bias=eps_t[:], scale=1.0,
                        )
                        nc.vector.reciprocal(out=mv[:, 1:2], in_=mv[:, 2:3])
                        for i in range(3):
                            nc.tensor.matmul(
                                statrow[
                                    32 * i:32 * i + 1,
                                    tt * 128:(tt + 1) * 128,
                                ],
                                lhsT=mv[:, i:i + 1], rhs=identity[:],
                                start=True, stop=True,
                            )
                        mvs.append(mv)
                    srow = srowp.tile([65, CHUNK], f32r, tag="srow")
                    with nc.allow_low_precision(reason="f32r matmul operand"):
                        nc.vector.tensor_copy(out=srow[:], in_=statrow[:])
                    rsb = rsbp.tile([128, CHUNK], f32, tag="rsb")
                    nc.tensor.matmul(
                        rsb[:], lhsT=ones_r[32:33, :], rhs=srow[32:33, :],
                        start=True, stop=True,
                    )
                    rsbs = rssp.tile([128, CHUNK], f32, tag="rsbs")
                    nc.scalar.activation(out=rsbs[:], in_=rsb[:], func=AF.Copy)
                    return srow, rsbs, mvs

                def load_slab(nm):
                    ws = wp.tile([128, KT, FPC], bf16, tag="wslab")
                    nc.sync.dma_start(out=ws[:], in_=w_r[nm])
                    return ws

                def gemm_block(nm, ws, xt_t, srow, rsb, mvs, c0):
                    flip = nm in ("q", "k")
                    pss = [
                        qpp.tile([128, CHUNK], f32, tag="pss", name=f"pss{f}")
                        for f in range(4)
                    ]
                    for f in range(4):
                        fs = slice(f * 128, (f + 1) * 128)
                        if flip:
                            # psum[d, tok] -= csum[d] * mu[tok]
                            nc.tensor.matmul(
                                pss[f][:], lhsT=ncs_c[nm][0:1, fs],
                                rhs=srow[0:1, :], start=True, stop=False,
                            )
                            if has_bias:
                                nc.tensor.matmul(
                                    pss[f][:], lhsT=b_c[nm][64:65, fs],
                                    rhs=srow[64:65, :], start=False, stop=False,
                                )
                        else:
                            # psum[tok, d] -= mu[tok] * csum[d]
                            nc.tensor.matmul(
                                pss[f][:], lhsT=srow[0:1, fs],
                                rhs=ncs_c[nm][0:1, :], start=True, stop=False,
                            )
                            if has_bias:
                                nc.tensor.matmul(
                                    pss[f][:], lhsT=srow[64:65, fs],
                                    rhs=b_c[nm][64:65, :], start=False, stop=False,
                                )
                    for kt in range(KT):
                        for f in range(4):
                            fs = slice(f * 128, (f + 1) * 128)
                            if flip:
                                nc.tensor.matmul(
                                    pss[f][:], lhsT=ws[:, kt, fs],
                                    rhs=xt_t[:, kt, :],
                                    start=False, stop=(kt == KT - 1),
                                )
                            else:
                                nc.tensor.matmul(
                                    pss[f][:], lhsT=xt_t[:, kt, fs],
                                    rhs=ws[:, kt, :],
                                    start=False, stop=(kt == KT - 1),
                                )
                    dst = {"q": qT_s, "k": kT_s}.get(nm)
                    for f in range(4):
                        st = stp.tile([128, CHUNK], bf16, tag="st", name=f"st{f}")
                        with nc.allow_low_precision(reason="bf16 scratch"):
                            if flip:
                                nc.vector.tensor_mul(
                                    out=st[:], in0=pss[f][:], in1=rsb[:]
                                )
                                nc.sync.dma_start(
                                    out=dst[f, :, c0:c0 + CHUNK], in_=st[:]
                                )
                            else:
                                nc.vector.tensor_scalar_mul(
                                    out=st[:], in0=pss[f][:],
                                    scalar1=mvs[f][:, 1:2],
                                )
                                nc.sync.dma_start(
                                    out=v_s[c0 + f * 128:c0 + (f + 1) * 128, :],
                                    in_=st[:],
                                )

                # prologue: chunk 0 inputs
                xt_ts = {}
                xt_ts[0] = xtp.tile(
                    [128, KT, CHUNK], bf16, tag="xts", name="xts0"
                )
                nc.sync.dma_start(out=xt_ts[0][:], in_=xt_r[:, :, 0:CHUNK])
                slab = {"q": load_slab("q")}
                stat = {0: emit_stats(0)}

                for c in range(NCHUNK):
                    c0 = c * CHUNK
                    if c + 1 < NCHUNK:
                        xt_ts[c + 1] = xtp.tile(
                            [128, KT, CHUNK], bf16, tag="xts",
                            name=f"xts{c + 1}",
                        )
                        nc.sync.dma_start(
                            out=xt_ts[c + 1][:],
                            in_=xt_r[:, :, c0 + CHUNK:c0 + 2 * CHUNK],
                        )
                    slab["k"] = load_slab("k")
                    if c + 1 < NCHUNK:
                        stat[c + 1] = emit_stats(c + 1)
                    srow, rsb, mvs = stat.pop(c)
                    xt_t = xt_ts.pop(c)
                    gemm_block("q", slab.pop("q"), xt_t, srow, rsb, mvs, c0)
                    slab["v"] = load_slab("v")
                    gemm_block("k", slab.pop("k"), xt_t, srow, rsb, mvs, c0)
                    if c + 1 < NCHUNK:
                        slab["q"] = load_slab("q")
                    gemm_block("v", slab.pop("v"), xt_t, srow, rsb, mvs, c0)

            # ------------- Phase B: attention -------------
            with tc.tile_pool(name="ctxp", bufs=1) as ctxp:
                ctx_t = [
                    ctxp.tile([128, S], bf16, tag=f"ctx{u}", name=f"ctx{u}")
                    for u in range(B * HPC)
                ]
                with tc.tile_pool(name="qtp", bufs=2) as qtp, \
                     tc.tile_pool(name="ktp", bufs=2) as ktp, \
                     tc.tile_pool(name="vp", bufs=2) as vp, \
                     tc.tile_pool(name="ep", bufs=6) as ep, \
                     tc.tile_pool(name="esp", bufs=6) as esp, \
                     tc.tile_pool(name="enp", bufs=2) as enp, \
                     tc.tile_pool(name="vnp", bufs=4) as vnp, \
                     tc.tile_pool(name="rp", bufs=3) as rp, \
                     tc.tile_pool(name="scp", bufs=3, space="PSUM") as scp, \
                     tc.tile_pool(name="cpp", bufs=2, space="PSUM") as cpp, \
                     tc.tile_pool(name="smp", bufs=2, space="PSUM") as smp, \
                     tc.tile_pool(name="mscp", bufs=1, space="PSUM") as mscp:
                    for u in range(B * HPC):
                        b, hh = divmod(u, HPC)
                        qt = qtp.tile([128, S], bf16)
                        nc.sync.dma_start(
                            out=qt[:], in_=qT_s[hh, :, b * S:(b + 1) * S]
                        )
                        kt_h = ktp.tile([128, S], bf16)
                        nc.sync.dma_start(
                            out=kt_h[:], in_=kT_s[hh, :, b * S:(b + 1) * S]
                        )
                        vt = vp.tile([128, S // 128, 128], bf16)
                        nc.sync.dma_start(
                            out=vt[:],
                            in_=v_s[
                                b * S:(b + 1) * S, hh * 128:(hh + 1) * 128
                            ].rearrange("(kj p) d -> p kj d", p=128),
                        )
                        # producer/consumer queue: consumers (es add, sums,
                        # PV, finalize) trail the score/exp producers by
                        # DEPTH entries so PE never waits on ACT in program
                        # order.
                        pend = []

                        def drain(n, pend=pend):
                            while len(pend) > n:
                                pend.pop(0)()

                        for qi in range(S // QTILE):
                            q0 = qi * QTILE
                            nkj = (q0 + QTILE) // 128
                            ctx_ps = cpp.tile(
                                [128, QTILE], f32, tag="ctxps",
                                name=f"ctxps{u}_{qi}",
                            )
                            sums = smp.tile(
                                [1, QTILE], f32, tag="sums",
                                name=f"sums{u}_{qi}",
                            )
                            ext = fm and qi == 0
                            for kjp in range(nkj // 2):
                                es2 = []
                                for j in range(2):
                                    kj = 2 * kjp + j
                                    sc = scp.tile(
                                        [128, QTILE], f32, tag="sc",
                                        name=f"sc{u}_{qi}_{kj}",
                                    )
                                    nc.tensor.matmul(
                                        sc[:],
                                        lhsT=kt_h[:, kj * 128:(kj + 1) * 128],
                                        rhs=qt[:, q0:q0 + QTILE],
                                        start=True, stop=True,
                                    )
                                    d = kj - (q0 // 128)
                                    if d >= 0:
                                        w = 128 * (d + 1)
                                        nc.vector.tensor_add(
                                            out=sc[:, 0:w],
                                            in0=sc[:, 0:w],
                                            in1=causal[:, d, 0:w],
                                        )
                                    e = ep.tile(
                                        [128, QTILE], bf16, tag="e",
                                        name=f"e{u}_{qi}_{kj}",
                                    )
                                    nc.scalar.activation(
                                        out=e[:], in_=sc[:], func=AF.Exp,
                                        bias=ab_c[:, u, kj:kj + 1], scale=1.0,
                                    )
                                    es2.append(e)

                                def consume(kjp=kjp, es2=es2, ctx_ps=ctx_ps,
                                            sums=sums, vt=vt,
                                            last=(kjp == nkj // 2 - 1)
                                            and not ext):
                                    es = esp.tile(
                                        [128, QTILE], bf16, tag="es"
                                    )
                                    with nc.allow_low_precision(
                                        reason="e pair add"
                                    ):
                                        nc.vector.tensor_add(
                                            out=es[:], in0=es2[0][:],
                                            in1=es2[1][:],
                                        )
                                    nc.tensor.matmul(
                                        sums[:], lhsT=ones_bf[:], rhs=es[:],
                                        start=(kjp == 0), stop=last,
                                    )
                                    nc.tensor.matmul(
                                        ctx_ps[:], lhsT=vt[:, 2 * kjp, :],
                                        rhs=es2[0][:],
                                        start=(kjp == 0), stop=False,
                                    )
                                    nc.tensor.matmul(
                                        ctx_ps[:], lhsT=vt[:, 2 * kjp + 1, :],
                                        rhs=es2[1][:],
                                        start=False, stop=last,
                                    )

                                pend.append(consume)
                                drain(2)
                            if ext:
                                # narrow pass-2: queries 0..127, kj 4..15,
                                # multiplicative key weights m2
                                for g in range(NP2 // 4):
                                    scn = mscp.tile(
                                        [128, 4, 128], f32, tag="msc",
                                        name=f"scn{u}_{g}",
                                    )
                                    en = enp.tile(
                                        [128, 4, 128], bf16, tag="en",
                                        name=f"en{u}_{g}",
                                    )
                                    for j in range(4):
                                        kj = 4 + g * 4 + j
                                        nc.tensor.matmul(
                                            scn[:, j, :],
                                            lhsT=kt_h[
                                                :, kj * 128:(kj + 1) * 128
                                            ],
                                            rhs=qt[:, 0:128],
                                            start=True, stop=True,
                                        )
                                    nc.scalar.activation(
                                        out=en[:], in_=scn[:], func=AF.Exp,
                                        scale=1.0,
                                    )

                                    def consume2(g=g, en=en, vt=vt, u=u,
                                                 ctx_ps=ctx_ps, sums=sums):
                                        for j in range(4):
                                            kj = 4 + g * 4 + j
                                            ki = g * 4 + j
                                            vtn = vnp.tile(
                                                [128, 128], bf16, tag="vtn"
                                            )
                                            with nc.allow_low_precision(
                                                reason="bf16 v scale"
                                            ):
                                                nc.vector.tensor_scalar_mul(
                                                    out=vtn[:],
                                                    in0=vt[:, kj, :],
                                                    scalar1=m2_c[
                                                        :, u, ki:ki + 1
                                                    ],
                                                )
                                            last2 = ki == NP2 - 1
                                            nc.tensor.matmul(
                                                sums[0:1, 0:128],
                                                lhsT=m2_cb[:, u, ki:ki + 1],
                                                rhs=en[:, j, :],
                                                start=False, stop=last2,
                                                skip_group_check=True,
                                            )
                                            nc.tensor.matmul(
                                                ctx_ps[:, 0:128], lhsT=vtn[:],
                                                rhs=en[:, j, :],
                                                start=False, stop=last2,
                                                skip_group_check=True,
                                            )

                                    pend.append(consume2)
                                    drain(2)

                            def finalize(u=u, qi=qi, q0=q0, ctx_ps=ctx_ps,
                                         sums=sums):
                                rcp = rp.tile([1, QTILE], f32r, tag="rcp")
                                with nc.allow_low_precision(
                                    reason="f32r matmul operand"
                                ):
                                    nc.vector.reciprocal(
                                        out=rcp[:], in_=sums[:]
                                    )
                                rsb2 = mscp.tile(
                                    [128, QTILE], f32, tag="msc",
                                    name=f"rsb2{u}_{qi}",
                                )
                                nc.tensor.matmul(
                                    rsb2[:], lhsT=ones_r[0:1, :], rhs=rcp[:],
                                    start=True, stop=True,
                                )
                                rsbs2 = rp.tile(
                                    [128, QTILE], f32, tag="rsbs2"
                                )
                                nc.vector.tensor_copy(
                                    out=rsbs2[:], in_=rsb2[:]
                                )
                                with nc.allow_low_precision(reason="bf16 ctx"):
                                    nc.vector.tensor_mul(
                                        out=ctx_t[u][:, q0:q0 + QTILE],
                                        in0=ctx_ps[:], in1=rsbs2[:],
                                    )

                            pend.append(finalize)
                            drain(3)
                        drain(0)

                # ------------- Phase C: out-proj -------------
                with tc.tile_pool(name="wop", bufs=2) as wop, \
                     tc.tile_pool(name="osp", bufs=6) as osp, \
                     tc.tile_pool(name="opp", bufs=4, space="PSUM") as opp:
                    for hs in range(H // 512):
                        wo_t = wop.tile([128, HPC, 512], bf16)
                        nc.sync.dma_start(
                            out=wo_t[:],
                            in_=wout[:, hs * 512:(hs + 1) * 512].rearrange(
                                "(f p) h -> p f h", p=128
                            ),
                        )
                        for ti in range(T // 128):
                            bb, tloc = divmod(ti, S // 128)
                            ps = opp.tile([128, 512], f32, tag="ops")
                            for f in range(HPC):
                                nc.tensor.matmul(
                                    ps[:],
                                    lhsT=ctx_t[bb * HPC + f][
                                        :, tloc * 128:(tloc + 1) * 128
                                    ],
                                    rhs=wo_t[:, f, :],
                                    start=(f == 0), stop=(f == HPC - 1),
                                )
                            ost = osp.tile([128, 512], f32, tag="ost")
                            nc.scalar.activation(
                                out=ost[:], in_=ps[:], func=AF.Copy
                            )
                            nc.sync.dma_start(
                                out=out[
                                    ti * 128:(ti + 1) * 128,
                                    hs * 512:(hs + 1) * 512,
                                ],
                                in_=ost[:],
                            )
    return nc


_NC_CACHE = {}
_LAST_CFG = (True, False)


def _get_nc(fm: bool = True, has_bias: bool = False):
    key = (fm, has_bias)
    if key not in _NC_CACHE:
        _NC_CACHE[key] = build_nc(fm, has_bias)
    return _NC_CACHE[key]


def _shard_inputs(x, input_mask, alibi, norm_w, norm_b, w_qkv, b_qkv, w_out, b_out):
    scale = np.float32(1.0 / np.sqrt(np.sqrt(np.float32(HD))))
    xf = np.ascontiguousarray(x.reshape(T, H), dtype=np.float32)
    x_bf = xf.astype(BF)
    xT_bf = np.ascontiguousarray(x_bf.T)
    nw = norm_w.astype(np.float32)
    nb = norm_b.astype(np.float32)
    mask_bias = (1.0 - input_mask.astype(np.float32)) * np.float32(NEG)  # [B, S]

    # fully-masked rows live only among the first lz = leading-masked-key
    # count queries; the narrow pass-2 covers queries 0..127.
    lz = 0
    for b in range(B):
        mb = input_mask[b]
        lz = max(lz, int(np.argmax(mb == 1)) if mb.any() else S)
    assert lz < 128, f"leading masked-key run {lz} >= 128 unsupported"
    fm = lz > 0

    in_maps = []
    for c in range(NCORES):
        sl_q = slice(c * FPC, (c + 1) * FPC)
        sl_k = slice(H + c * FPC, H + (c + 1) * FPC)
        sl_v = slice(2 * H + c * FPC, 2 * H + (c + 1) * FPC)
        wq_c = ((nw[:, None] * w_qkv[:, sl_q]) * scale).astype(BF)
        wk_c = ((nw[:, None] * w_qkv[:, sl_k]) * scale).astype(BF)
        wv_c = (nw[:, None] * w_qkv[:, sl_v]).astype(BF)
        # negated column sums over the *rounded* weights (exact mean fold)
        ncsq = -wq_c.astype(np.float32).sum(0)[None, :]
        ncsk = -wk_c.astype(np.float32).sum(0)[None, :]
        ncsv = -wv_c.astype(np.float32).sum(0)[None, :]
        bq_c = ((b_qkv[sl_q] + nb @ w_qkv[:, sl_q]) * scale)[None, :]
        bk_c = ((b_qkv[sl_k] + nb @ w_qkv[:, sl_k]) * scale)[None, :]
        bv_c = (b_qkv[sl_v] + nb @ w_qkv[:, sl_v])[None, :]
        has_bias = bool(
            np.abs(bq_c).max() > 0 or np.abs(bk_c).max() > 0
            or np.abs(bv_c).max() > 0
        )
        ab = np.empty((B * HPC, S), np.float32)
        for b in range(B):
            for hh in range(HPC):
                ab[b * HPC + hh] = alibi[c * HPC + hh, 0, :] + mask_bias[b]
        ab_t = np.ascontiguousarray(
            ab.reshape(B * HPC, S // 128, 128).transpose(2, 0, 1)
        )
        m2_t = np.exp(ab_t[:, :, 4:16].astype(np.float64) + NEG).astype(np.float32)
        im = {
            "xr": x_bf,
            "xt": xT_bf,
            "wq": np.ascontiguousarray(wq_c),
            "wk": np.ascontiguousarray(wk_c),
            "wv": np.ascontiguousarray(wv_c),
            "ncsq": np.ascontiguousarray(ncsq, np.float32),
            "ncsk": np.ascontiguousarray(ncsk, np.float32),
            "ncsv": np.ascontiguousarray(ncsv, np.float32),
            "abias": ab_t,
            "wout": np.ascontiguousarray(w_out[sl_q, :].astype(BF)),
        }
        if has_bias:
            im["bqr"] = np.ascontiguousarray(bq_c, np.float32)
            im["bkr"] = np.ascontiguousarray(bk_c, np.float32)
            im["bvr"] = np.ascontiguousarray(bv_c, np.float32)
        if fm:
            im["m2"] = np.ascontiguousarray(m2_t)
        in_maps.append(im)
    return in_maps, fm, has_bias


def kernel(x, input_mask, alibi, norm_w, norm_b, w_qkv, b_qkv, w_out, b_out):
    from concourse.bass_utils import run_bass_kernel_spmd

    in_maps, fm, has_bias = _shard_inputs(
        np.asarray(x), np.asarray(input_mask), np.asarray(alibi),
        np.asarray(norm_w), np.asarray(norm_b), np.asarray(w_qkv),
        np.asarray(b_qkv), np.asarray(w_out), np.asarray(b_out),
    )
    global _LAST_CFG
    _LAST_CFG = (fm, has_bias)
    nc = _get_nc(fm, has_bias)
    res = run_bass_kernel_spmd(nc, in_maps, core_ids=list(range(NCORES)))
    acc = res.results[0]["out"].astype(np.float32).copy()
    for c in range(1, NCORES):
        acc += res.results[c]["out"]
    acc += np.asarray(b_out, np.float32)[None, :]
    return acc.reshape(B, S, H)


# revision 16
# speedup vs baseline: 1.0821x; 1.0232x over previous
"""DeepSpeed-style self-attention block on 8 Trainium2 NeuronCores.

Tensor-parallel over heads (4 heads/core), DeepSpeed mp_size=8 style:
  - w_qkv column-sharded [H, 3H/8] (per-core wq/wk/wv [H, 512], bf16)
  - w_out row-sharded   [H/8, H]  -> per-core partial outputs
  - layernorm stats computed on device; partial-sum reduction + b_out on host.

Device kernel structure (per core, identical SPMD program, sharded inputs):
  Phase A: the host passes x twice in bf16 - row-major (for LN stats) and
    pre-transposed xT [H, T] (pure layout change).  Per 512-token chunk:
    DVE bn_stats/bn_aggr give mean/rstd/std columns; a tiny PE matmul
    transposes them to rows.  The QKV gemms run directly on RAW xT
    (bf16, full rate); layernorm is folded in algebraically:
       qT = ((W^T xT) - csum(W) (x) mu + b (x) std) * rstd[t]
    The two rank-1 terms are extra K=1 matmuls accumulated into the same
    PSUM; the per-token rstd scale is one DVE multiply against a
    PE-broadcast rstd row (q/k outputs, free-dim tokens) or a per-partition
    tensor_scalar (v output, partition-dim tokens).  norm_w / the
    1/sqrt(sqrt(hd)) scale are folded into the bf16 weights on the host;
    column sums are taken over the *rounded* bf16 weights so the mean
    correction is exact.  Weights stream as one 4MB slab DMA per
    (weight, chunk).  qT/kT [d, tok] and v [tok, d] go to DRAM scratch in
    bf16.
  Phase B: per (batch, head): scoresT = kT^T @ qT -> [k, q] psum blocks
    (two 512-wide kj blocks per 2-bank psum tile); causal handled as
    additive -50 tiles on the diagonal-straddling blocks only (DVE);
    exp via ACT with per-key (mask+alibi) bias; pairs of e tiles are
    pre-added on DVE so the row-sum ones-matmul runs once per 2 kj;
    PV matmul accumulates ctxT [d, q]; 1/sum broadcast via K=1 matmul,
    applied on the PSUM->SBUF ctx write (DVE).
    Rows that DeepSpeed's tied -10000 constants make attend over the
    *whole* sequence (all their visible keys input-masked) can only live
    among the first <128 queries (host-verified); a narrow pass-2 covers
    queries 0..127 over kj 4..15 with multiplicative key weights
    m2 = exp(alibi + maskbias + NEG), accumulated into the same psum.
    (-50 instead of DeepSpeed's -10000 keeps exp() in fp32 range; softmax
    is shift-invariant and masked weights come out ~1e-19, matching the
    reference's exact 0s well below tolerance.)
  Phase C: out-proj partial: out += ctxT^T @ w_out_shard per token tile.

All big matmuls run in bf16 (full rate); accumulation is f32 PSUM.
The walrus build here allows only ONE semaphore wait per instruction;
PatchedTileContext splits surplus Tile-emitted waits onto NoOps.
"""

import numpy as np
import ml_dtypes

import concourse.bass as bass
import concourse.mybir as mybir
import concourse.tile as tile
from concourse import masks

f32 = mybir.dt.float32
f32r = mybir.dt.float32r
bf16 = mybir.dt.bfloat16
BF = ml_dtypes.bfloat16

B, S, H, NH = 2, 2048, 4096, 32
HD = H // NH            # 128 head dim
NCORES = 8
HPC = NH // NCORES      # 4 heads per core
FPC = HPC * HD          # 512 sharded features per core
T = B * S               # 4096 tokens
KT = H // 128           # 32 contraction tiles
CHUNK = 512             # tokens per QKV chunk
NCHUNK = T // CHUNK     # 8
QTILE = 512             # query block in attention
NP2 = 12                # pass-2 kj blocks (kj 4..15) for fully-masked rows
LN_EPS = 1e-5
NEG = -50.0             # soft mask value (see module docstring)


class PatchedTileContext(tile.TileContext):
    """This container's walrus build rejects >1 sync-wait per instruction;
    split surplus waits onto preceding same-engine NoOps."""

    _wsplit_n = 0

    def _commit_instruction(self, inst, lazy_reg_writes: bool = True):
        si = inst.sync_info
        if si is not None and si.on_wait and len(si.on_wait) > 1:
            waits = list(si.on_wait)
            inst.sync_info = mybir.SyncInfo(
                on_wait=[waits[-1]], on_update=list(si.on_update or [])
            )
            for w in waits[:-1]:
                type(self)._wsplit_n += 1
                n = mybir.InstNoOp(name=f"wsplit-{type(self)._wsplit_n}")
                n.engine = inst.engine
                n.sync_info = mybir.SyncInfo(on_wait=[w], on_update=[])
                self._add_instruction(n)
        return super()._commit_instruction(inst, lazy_reg_writes)

    def _drain_and_barrier(self, tick_clock, wait_clock):
        from concourse.vector_clock import ScopedClock

        nc = self.nc
        collector = nc.sync.nop(nofuse=True)
        wait_clock.add_sem_waits(
            collector.ins, ScopedClock({None: tick_clock.global_clock})
        )
        waits = list(collector.ins.sync_info.on_wait)
        collector.ins.sync_info = mybir.SyncInfo(on_wait=[], on_update=[])
        for w in waits:
            n = nc.sync.nop(nofuse=True)
            n.ins.sync_info = mybir.SyncInfo(on_wait=[w], on_update=[])
        nc.sync.drain()
        nc.all_engine_barrier()
        assert self.sems is not None
        popped = nc._tile_sem_poison_stack.pop()
        assert popped is self._sem_poison
        nc.clear_and_free_semaphores(list(self.sems.allocated().values()))
        nc.all_engine_barrier()


AF = mybir.ActivationFunctionType


def build_nc(fm: bool = True, has_bias: bool = False):
    nc = bass.Bass(target_bir_lowering=False)

    xr = nc.declare_dram_parameter("xr", [T, H], bf16, isOutput=False).ap()
    xt = nc.declare_dram_parameter("xt", [H, T], bf16, isOutput=False).ap()
    wq = nc.declare_dram_parameter("wq", [H, FPC], bf16, isOutput=False).ap()
    wk = nc.declare_dram_parameter("wk", [H, FPC], bf16, isOutput=False).ap()
    wv = nc.declare_dram_parameter("wv", [H, FPC], bf16, isOutput=False).ap()
    # negated column sums of the (bf16-rounded) weights, and bias rows
    ncsq = nc.declare_dram_parameter("ncsq", [1, FPC], f32r, isOutput=False).ap()
    ncsk = nc.declare_dram_parameter("ncsk", [1, FPC], f32r, isOutput=False).ap()
    ncsv = nc.declare_dram_parameter("ncsv", [1, FPC], f32r, isOutput=False).ap()
    if has_bias:
        bqr = nc.declare_dram_parameter("bqr", [1, FPC], f32r, isOutput=False).ap()
        bkr = nc.declare_dram_parameter("bkr", [1, FPC], f32r, isOutput=False).ap()
        bvr = nc.declare_dram_parameter("bvr", [1, FPC], f32r, isOutput=False).ap()
    abias = nc.declare_dram_parameter(
        "abias", [128, B * HPC, S // 128], f32, isOutput=False
    ).ap()
    if fm:
        # multiplicative key weights exp(abias + NEG) for the narrow pass-2
        m2 = nc.declare_dram_parameter(
            "m2", [128, B * HPC, NP2], f32, isOutput=False
        ).ap()
    wout = nc.declare_dram_parameter("wout", [FPC, H], bf16, isOutput=False).ap()
    out = nc.declare_dram_parameter("out", [T, H], f32, isOutput=True).ap()

    # DRAM scratch
    qT_s = nc.dram_tensor("qT_s", [HPC, 128, T], bf16).ap()
    kT_s = nc.dram_tensor("kT_s", [HPC, 128, T], bf16).ap()
    v_s = nc.dram_tensor("v_s", [T, FPC], bf16).ap()

    xt_r = xt.rearrange("(kt p) t -> p kt t", p=128)
    w_r = {
        "q": wq.rearrange("(kt p) f -> p kt f", p=128),
        "k": wk.rearrange("(kt p) f -> p kt f", p=128),
        "v": wv.rearrange("(kt p) f -> p kt f", p=128),
    }

    with PatchedTileContext(nc) as tc:
        with tc.tile_pool(name="singles", bufs=1) as singles:
            identity = singles.tile([128, 128], f32)
            masks.make_identity(nc, identity[:])
            ones_f = singles.tile([128, 128], f32)
            nc.vector.memset(ones_f[:], 1.0)
            ones_r = singles.tile([128, 128], f32r)
            nc.scalar.activation(out=ones_r[:], in_=ones_f[:], func=AF.Copy)
            ones_bf = singles.tile([128, 1], bf16)
            with nc.allow_low_precision(reason="exact ones"):
                nc.vector.tensor_copy(out=ones_bf[:], in_=ones_f[:, 0:1])
            eps_t = singles.tile([128, 1], f32)
            nc.vector.memset(eps_t[:], LN_EPS)
            # additive causal tiles, one per diagonal offset d = (k0-q0)/128
            causal = singles.tile([128, 4, QTILE], f32)
            nc.gpsimd.memset(causal[:], 0.0)
            for d in range(4):
                nc.gpsimd.affine_select(
                    out=causal[:, d, :],
                    in_=causal[:, d, :],
                    compare_op=mybir.AluOpType.is_ge,
                    fill=NEG,
                    base=-(128 * d),
                    pattern=[[1, QTILE]],
                    channel_multiplier=-1,
                )
            # row constants (csum / bias rows)
            ncs_c = {}
            for nm, src in (("q", ncsq), ("k", ncsk), ("v", ncsv)):
                t = singles.tile([1, FPC], f32r, name=f"ncs_{nm}")
                nc.gpsimd.dma_start(out=t[:], in_=src)
                ncs_c[nm] = t
            b_c = {}
            if has_bias:
                for nm, bsrc in (("q", bqr), ("k", bkr), ("v", bvr)):
                    t = singles.tile([65, FPC], f32r, name=f"b_{nm}")
                    nc.gpsimd.dma_start(out=t[64:65, :], in_=bsrc)
                    b_c[nm] = t
            ab_c = singles.tile([128, B * HPC, S // 128], f32)
            nc.gpsimd.dma_start(out=ab_c[:], in_=abias)
            if fm:
                m2_c = singles.tile([128, B * HPC, NP2], f32)
                nc.gpsimd.dma_start(out=m2_c[:], in_=m2)
                m2_cb = singles.tile([128, B * HPC, NP2], bf16)
                nc.scalar.activation(out=m2_cb[:], in_=m2_c[:], func=AF.Copy)

            # ---------------- Phase A: LN stats + QKV gemms ----------------
            with tc.tile_pool(name="xrp", bufs=4) as xrp, \
                 tc.tile_pool(name="statp", bufs=4) as statp, \
                 tc.tile_pool(name="mvp", bufs=8) as mvp, \
                 tc.tile_pool(name="srowp", bufs=2) as srowp, \
                 tc.tile_pool(name="rssp", bufs=2) as rssp, \
                 tc.tile_pool(name="xtp", bufs=2) as xtp, \
                 tc.tile_pool(name="wp", bufs=2) as wp, \
                 tc.tile_pool(name="stp", bufs=6) as stp, \
                 tc.tile_pool(name="strp", bufs=1, space="PSUM") as strp, \
                 tc.tile_pool(name="rsbp", bufs=2, space="PSUM") as rsbp, \
                 tc.tile_pool(name="qpp", bufs=5, space="PSUM") as qpp:

                def emit_stats(c):
                    """LN stats for chunk c: returns (srow, rsb, mv tiles).
                    srow [65, CHUNK] f32r rows at PE-legal base partitions:
                    0=mean, 32=rstd, 64=std.
                    rsb = rstd broadcast [128, CHUNK] (PSUM f32)."""
                    statrow = strp.tile([65, CHUNK], f32, tag="statrow")
                    mvs = []
                    for tt in range(CHUNK // 128):
                        g = c * (CHUNK // 128) + tt
                        xr_t = xrp.tile([128, H], bf16)
                        nc.sync.dma_start(
                            out=xr_t[:], in_=xr[g * 128:(g + 1) * 128, :]
                        )
                        stats = statp.tile([128, H // 512, 6], f32)
                        xg = xr_t[:].rearrange("p (n f) -> p n f", f=512)
                        for n in range(H // 512):
                            nc.vector.bn_stats(out=stats[:, n, :], in_=xg[:, n, :])
                        mv = mvp.tile([128, 3], f32)
                        nc.vector.bn_aggr(out=mv[:, 0:2], in_=stats[:])
                        nc.scalar.activation(
                            out=mv[:, 2:3], in_=mv[:, 1:2], func=AF.Sqrt,
                            bias=eps_t[:], scale=1.0,
                        )
                        nc.vector.reciprocal(out=mv[:, 1:2], in_=mv[:, 2:3])
                        for i in range(3):
                            nc.tensor.matmul(
                                statrow[
                                    32 * i:32 * i + 1,
                                    tt * 128:(tt + 1) * 128,
                                ],
                                lhsT=mv[:, i:i + 1], rhs=identity[:],
                                start=True, stop=True,
                            )
                        mvs.append(mv)
                    srow = srowp.tile([65, CHUNK], f32r, tag="srow")
                    with nc.allow_low_precision(reason="f32r matmul operand"):
                        nc.vector.tensor_copy(out=srow[:], in_=statrow[:])
                    rsb = rsbp.tile([128, CHUNK], f32, tag="rsb")
                    nc.tensor.matmul(
                        rsb[:], lhsT=ones_r[32:33, :], rhs=srow[32:33, :],
                        start=True, stop=True,
                    )
                    rsbs = rssp.tile([128, CHUNK], f32, tag="rsbs")
                    nc.scalar.activation(out=rsbs[:], in_=rsb[:], func=AF.Copy)
                    return srow, rsbs, mvs

                def load_slab(nm, pieces=2):
                    ws = wp.tile([128, KT, FPC], bf16, tag="wslab")
                    step = KT // pieces
                    for p in range(pieces):
                        nc.sync.dma_start(
                            out=ws[:, p * step:(p + 1) * step, :],
                            in_=w_r[nm][:, p * step:(p + 1) * step, :],
                        )
                    return ws

                def gemm_block(nm, ws, xt_t, srow, rsb, mvs, c0):
                    flip = nm in ("q", "k")
                    pss = [
                        qpp.tile([128, CHUNK], f32, tag="pss", name=f"pss{f}")
                        for f in range(4)
                    ]
                    for f in range(4):
                        fs = slice(f * 128, (f + 1) * 128)
                        if flip:
                            # psum[d, tok] -= csum[d] * mu[tok]
                            nc.tensor.matmul(
                                pss[f][:], lhsT=ncs_c[nm][0:1, fs],
                                rhs=srow[0:1, :], start=True, stop=False,
                            )
                            if has_bias:
                                nc.tensor.matmul(
                                    pss[f][:], lhsT=b_c[nm][64:65, fs],
                                    rhs=srow[64:65, :], start=False, stop=False,
                                )
                        else:
                            # psum[tok, d] -= mu[tok] * csum[d]
                            nc.tensor.matmul(
                                pss[f][:], lhsT=srow[0:1, fs],
                                rhs=ncs_c[nm][0:1, :], start=True, stop=False,
                            )
                            if has_bias:
                                nc.tensor.matmul(
                                    pss[f][:], lhsT=srow[64:65, fs],
                                    rhs=b_c[nm][64:65, :], start=False, stop=False,
                                )
                    for kt in range(KT):
                        for f in range(4):
                            fs = slice(f * 128, (f + 1) * 128)
                            if flip:
                                nc.tensor.matmul(
                                    pss[f][:], lhsT=ws[:, kt, fs],
                                    rhs=xt_t[:, kt, :],
                                    start=False, stop=(kt == KT - 1),
                                )
                            else:
                                nc.tensor.matmul(
                                    pss[f][:], lhsT=xt_t[:, kt, fs],
                                    rhs=ws[:, kt, :],
                                    start=False, stop=(kt == KT - 1),
                                )
                    dst = {"q": qT_s, "k": kT_s}.get(nm)
                    for f in range(4):
                        st = stp.tile([128, CHUNK], bf16, tag="st", name=f"st{f}")
                        with nc.allow_low_precision(reason="bf16 scratch"):
                            if flip:
                                nc.vector.tensor_mul(
                                    out=st[:], in0=pss[f][:], in1=rsb[:]
                                )
                                nc.sync.dma_start(
                                    out=dst[f, :, c0:c0 + CHUNK], in_=st[:]
                                )
                            else:
                                nc.vector.tensor_scalar_mul(
                                    out=st[:], in0=pss[f][:],
                                    scalar1=mvs[f][:, 1:2],
                                )
                                nc.sync.dma_start(
                                    out=v_s[c0 + f * 128:c0 + (f + 1) * 128, :],
                                    in_=st[:],
                                )

                # prologue: stats DMAs first (smallest critical path),
                # then weight/xT slabs in quarters so the first gemms can
                # start on partial slabs.
                xt_ts = {}
                stat = {0: emit_stats(0)}
                xt_ts[0] = xtp.tile(
                    [128, KT, CHUNK], bf16, tag="xts", name="xts0"
                )
                slab = {"q": wp.tile([128, KT, FPC], bf16, tag="wslab",
                                     name="wq0")}
                for p in range(4):
                    st8 = slice(p * (KT // 4), (p + 1) * (KT // 4))
                    nc.sync.dma_start(
                        out=slab["q"][:, st8, :], in_=w_r["q"][:, st8, :]
                    )
                    nc.sync.dma_start(
                        out=xt_ts[0][:, st8, :],
                        in_=xt_r[:, st8, 0:CHUNK],
                    )

                for c in range(NCHUNK):
                    c0 = c * CHUNK
                    if c + 1 < NCHUNK:
                        xt_ts[c + 1] = xtp.tile(
                            [128, KT, CHUNK], bf16, tag="xts",
                            name=f"xts{c + 1}",
                        )
                        for p in range(2):
                            sth = slice(p * (KT // 2), (p + 1) * (KT // 2))
                            nc.sync.dma_start(
                                out=xt_ts[c + 1][:, sth, :],
                                in_=xt_r[:, sth, c0 + CHUNK:c0 + 2 * CHUNK],
                            )
                    slab["k"] = load_slab("k")
                    if c + 1 < NCHUNK:
                        stat[c + 1] = emit_stats(c + 1)
                    srow, rsb, mvs = stat.pop(c)
                    xt_t = xt_ts.pop(c)
                    gemm_block("q", slab.pop("q"), xt_t, srow, rsb, mvs, c0)
                    slab["v"] = load_slab("v")
                    gemm_block("k", slab.pop("k"), xt_t, srow, rsb, mvs, c0)
                    if c + 1 < NCHUNK:
                        slab["q"] = load_slab("q")
                    gemm_block("v", slab.pop("v"), xt_t, srow, rsb, mvs, c0)

            # ------------- Phase B: attention -------------
            with tc.tile_pool(name="ctxp", bufs=1) as ctxp:
                ctx_t = [
                    ctxp.tile([128, S], bf16, tag=f"ctx{u}", name=f"ctx{u}")
                    for u in range(B * HPC)
                ]
                with tc.tile_pool(name="qtp", bufs=2) as qtp, \
                     tc.tile_pool(name="ktp", bufs=2) as ktp, \
                     tc.tile_pool(name="vp", bufs=2) as vp, \
                     tc.tile_pool(name="ep", bufs=6) as ep, \
                     tc.tile_pool(name="esp", bufs=6) as esp, \
                     tc.tile_pool(name="enp", bufs=2) as enp, \
                     tc.tile_pool(name="vnp", bufs=4) as vnp, \
                     tc.tile_pool(name="rp", bufs=3) as rp, \
                     tc.tile_pool(name="scp", bufs=3, space="PSUM") as scp, \
                     tc.tile_pool(name="cpp", bufs=2, space="PSUM") as cpp, \
                     tc.tile_pool(name="smp", bufs=2, space="PSUM") as smp, \
                     tc.tile_pool(name="mscp", bufs=1, space="PSUM") as mscp:
                    for u in range(B * HPC):
                        b, hh = divmod(u, HPC)
                        qt = qtp.tile([128, S], bf16)
                        nc.sync.dma_start(
                            out=qt[:], in_=qT_s[hh, :, b * S:(b + 1) * S]
                        )
                        kt_h = ktp.tile([128, S], bf16)
                        nc.sync.dma_start(
                            out=kt_h[:], in_=kT_s[hh, :, b * S:(b + 1) * S]
                        )
                        vt = vp.tile([128, S // 128, 128], bf16)
                        nc.sync.dma_start(
                            out=vt[:],
                            in_=v_s[
                                b * S:(b + 1) * S, hh * 128:(hh + 1) * 128
                            ].rearrange("(kj p) d -> p kj d", p=128),
                        )
                        # producer/consumer queue: consumers (es add, sums,
                        # PV, finalize) trail the score/exp producers by
                        # DEPTH entries so PE never waits on ACT in program
                        # order.
                        pend = []

                        def drain(n, pend=pend):
                            while len(pend) > n:
                                pend.pop(0)()

                        for qi in range(S // QTILE):
                            q0 = qi * QTILE
                            nkj = (q0 + QTILE) // 128
                            ctx_ps = cpp.tile(
                                [128, QTILE], f32, tag="ctxps",
                                name=f"ctxps{u}_{qi}",
                            )
                            sums = smp.tile(
                                [1, QTILE], f32, tag="sums",
                                name=f"sums{u}_{qi}",
                            )
                            ext = fm and qi == 0
                            for kjp in range(nkj // 2):
                                es2 = []
                                for j in range(2):
                                    kj = 2 * kjp + j
                                    sc = scp.tile(
                                        [128, QTILE], f32, tag="sc",
                                        name=f"sc{u}_{qi}_{kj}",
                                    )
                                    nc.tensor.matmul(
                                        sc[:],
                                        lhsT=kt_h[:, kj * 128:(kj + 1) * 128],
                                        rhs=qt[:, q0:q0 + QTILE],
                                        start=True, stop=True,
                                    )
                                    d = kj - (q0 // 128)
                                    if d >= 0:
                                        w = 128 * (d + 1)
                                        nc.vector.tensor_add(
                                            out=sc[:, 0:w],
                                            in0=sc[:, 0:w],
                                            in1=causal[:, d, 0:w],
                                        )
                                    e = ep.tile(
                                        [128, QTILE], bf16, tag="e",
                                        name=f"e{u}_{qi}_{kj}",
                                    )
                                    nc.scalar.activation(
                                        out=e[:], in_=sc[:], func=AF.Exp,
                                        bias=ab_c[:, u, kj:kj + 1], scale=1.0,
                                    )
                                    es2.append(e)

                                def consume(kjp=kjp, es2=es2, ctx_ps=ctx_ps,
                                            sums=sums, vt=vt,
                                            last=(kjp == nkj // 2 - 1)
                                            and not ext):
                                    es = esp.tile(
                                        [128, QTILE], bf16, tag="es"
                                    )
                                    with nc.allow_low_precision(
                                        reason="e pair add"
                                    ):
                                        nc.vector.tensor_add(
                                            out=es[:], in0=es2[0][:],
                                            in1=es2[1][:],
                                        )
                                    nc.tensor.matmul(
                                        sums[:], lhsT=ones_bf[:], rhs=es[:],
                                        start=(kjp == 0), stop=last,
                                    )
                                    nc.tensor.matmul(
                                        ctx_ps[:], lhsT=vt[:, 2 * kjp, :],
                                        rhs=es2[0][:],
                                        start=(kjp == 0), stop=False,
                                    )
                                    nc.tensor.matmul(
                                        ctx_ps[:], lhsT=vt[:, 2 * kjp + 1, :],
                                        rhs=es2[1][:],
                                        start=False, stop=last,
                                    )

                                pend.append(consume)
                                drain(2)
                            if ext:
                                # narrow pass-2: queries 0..127, kj 4..15,
                                # multiplicative key weights m2
                                for g in range(NP2 // 4):
                                    scn = mscp.tile(
                                        [128, 4, 128], f32, tag="msc",
                                        name=f"scn{u}_{g}",
                                    )
                                    en = enp.tile(
                                        [128, 4, 128], bf16, tag="en",
                                        name=f"en{u}_{g}",
                                    )
                                    for j in range(4):
                                        kj = 4 + g * 4 + j
                                        nc.tensor.matmul(
                                            scn[:, j, :],
                                            lhsT=kt_h[
                                                :, kj * 128:(kj + 1) * 128
                                            ],
                                            rhs=qt[:, 0:128],
                                            start=True, stop=True,
                                        )
                                    nc.scalar.activation(
                                        out=en[:], in_=scn[:], func=AF.Exp,
                                        scale=1.0,
                                    )

                                    def consume2(g=g, en=en, vt=vt, u=u,
                                                 ctx_ps=ctx_ps, sums=sums):
                                        for j in range(4):
                                            kj = 4 + g * 4 + j
                                            ki = g * 4 + j
                                            vtn = vnp.tile(
                                                [128, 128], bf16, tag="vtn"
                                            )
                                            with nc.allow_low_precision(
                                                reason="bf16 v scale"
                                            ):
                                                nc.vector.tensor_scalar_mul(
                                                    out=vtn[:],
                                                    in0=vt[:, kj, :],
                                                    scalar1=m2_c[
                                                        :, u, ki:ki + 1
                                                    ],
                                                )
                                            last2 = ki == NP2 - 1
                                            nc.tensor.matmul(
                                                sums[0:1, 0:128],
                                                lhsT=m2_cb[:, u, ki:ki + 1],
                                                rhs=en[:, j, :],
                                                start=False, stop=last2,
                                                skip_group_check=True,
                                            )
                                            nc.tensor.matmul(
                                                ctx_ps[:, 0:128], lhsT=vtn[:],
                                                rhs=en[:, j, :],
                                                start=False, stop=last2,
                                                skip_group_check=True,
                                            )

                                    pend.append(consume2)
                                    drain(2)

                            def finalize(u=u, qi=qi, q0=q0, ctx_ps=ctx_ps,
                                         sums=sums):
                                rcp = rp.tile([1, QTILE], f32r, tag="rcp")
                                with nc.allow_low_precision(
                                    reason="f32r matmul operand"
                                ):
                                    nc.vector.reciprocal(
                                        out=rcp[:], in_=sums[:]
                                    )
                                rsb2 = mscp.tile(
                                    [128, QTILE], f32, tag="msc",
                                    name=f"rsb2{u}_{qi}",
                                )
                                nc.tensor.matmul(
                                    rsb2[:], lhsT=ones_r[0:1, :], rhs=rcp[:],
                                    start=True, stop=True,
                                )
                                rsbs2 = rp.tile(
                                    [128, QTILE], f32, tag="rsbs2"
                                )
                                nc.vector.tensor_copy(
                                    out=rsbs2[:], in_=rsb2[:]
                                )
                                with nc.allow_low_precision(reason="bf16 ctx"):
                                    nc.vector.tensor_mul(
                                        out=ctx_t[u][:, q0:q0 + QTILE],
                                        in0=ctx_ps[:], in1=rsbs2[:],
                                    )

                            pend.append(finalize)
                            drain(3)
                        drain(0)

                # ------------- Phase C: out-proj -------------
                with tc.tile_pool(name="wop", bufs=2) as wop, \
                     tc.tile_pool(name="osp", bufs=6) as osp, \
                     tc.tile_pool(name="opp", bufs=4, space="PSUM") as opp:
                    for hs in range(H // 512):
                        wo_t = wop.tile([128, HPC, 512], bf16)
                        nc.sync.dma_start(
                            out=wo_t[:],
                            in_=wout[:, hs * 512:(hs + 1) * 512].rearrange(
                                "(f p) h -> p f h", p=128
                            ),
                        )
                        for ti in range(T // 128):
                            bb, tloc = divmod(ti, S // 128)
                            ps = opp.tile([128, 512], f32, tag="ops")
                            for f in range(HPC):
                                nc.tensor.matmul(
                                    ps[:],
                                    lhsT=ctx_t[bb * HPC + f][
                                        :, tloc * 128:(tloc + 1) * 128
                                    ],
                                    rhs=wo_t[:, f, :],
                                    start=(f == 0), stop=(f == HPC - 1),
                                )
                            ost = osp.tile([128, 512], f32, tag="ost")
                            nc.scalar.activation(
                                out=ost[:], in_=ps[:], func=AF.Copy
                            )
                            nc.sync.dma_start(
                                out=out[
                                    ti * 128:(ti + 1) * 128,
                                    hs * 512:(hs + 1) * 512,
                                ],
                                in_=ost[:],
                            )
    return nc


_NC_CACHE = {}
_LAST_CFG = (True, False)


def _get_nc(fm: bool = True, has_bias: bool = False):
    key = (fm, has_bias)
    if key not in _NC_CACHE:
        _NC_CACHE[key] = build_nc(fm, has_bias)
    return _NC_CACHE[key]


def _shard_inputs(x, input_mask, alibi, norm_w, norm_b, w_qkv, b_qkv, w_out, b_out):
    scale = np.float32(1.0 / np.sqrt(np.sqrt(np.float32(HD))))
    xf = np.ascontiguousarray(x.reshape(T, H), dtype=np.float32)
    x_bf = xf.astype(BF)
    xT_bf = np.ascontiguousarray(x_bf.T)
    nw = norm_w.astype(np.float32)
    nb = norm_b.astype(np.float32)
    mask_bias = (1.0 - input_mask.astype(np.float32)) * np.float32(NEG)  # [B, S]

    # fully-masked rows live only among the first lz = leading-masked-key
    # count queries; the narrow pass-2 covers queries 0..127.
    lz = 0
    for b in range(B):
        mb = input_mask[b]
        lz = max(lz, int(np.argmax(mb == 1)) if mb.any() else S)
    assert lz < 128, f"leading masked-key run {lz} >= 128 unsupported"
    fm = lz > 0

    in_maps = []
    for c in range(NCORES):
        sl_q = slice(c * FPC, (c + 1) * FPC)
        sl_k = slice(H + c * FPC, H + (c + 1) * FPC)
        sl_v = slice(2 * H + c * FPC, 2 * H + (c + 1) * FPC)
        wq_c = ((nw[:, None] * w_qkv[:, sl_q]) * scale).astype(BF)
        wk_c = ((nw[:, None] * w_qkv[:, sl_k]) * scale).astype(BF)
        wv_c = (nw[:, None] * w_qkv[:, sl_v]).astype(BF)
        # negated column sums over the *rounded* weights (exact mean fold)
        ncsq = -wq_c.astype(np.float32).sum(0)[None, :]
        ncsk = -wk_c.astype(np.float32).sum(0)[None, :]
        ncsv = -wv_c.astype(np.float32).sum(0)[None, :]
        bq_c = ((b_qkv[sl_q] + nb @ w_qkv[:, sl_q]) * scale)[None, :]
        bk_c = ((b_qkv[sl_k] + nb @ w_qkv[:, sl_k]) * scale)[None, :]
        bv_c = (b_qkv[sl_v] + nb @ w_qkv[:, sl_v])[None, :]
        has_bias = bool(
            np.abs(bq_c).max() > 0 or np.abs(bk_c).max() > 0
            or np.abs(bv_c).max() > 0
        )
        ab = np.empty((B * HPC, S), np.float32)
        for b in range(B):
            for hh in range(HPC):
                ab[b * HPC + hh] = alibi[c * HPC + hh, 0, :] + mask_bias[b]
        ab_t = np.ascontiguousarray(
            ab.reshape(B * HPC, S // 128, 128).transpose(2, 0, 1)
        )
        m2_t = np.exp(ab_t[:, :, 4:16].astype(np.float64) + NEG).astype(np.float32)
        im = {
            "xr": x_bf,
            "xt": xT_bf,
            "wq": np.ascontiguousarray(wq_c),
            "wk": np.ascontiguousarray(wk_c),
            "wv": np.ascontiguousarray(wv_c),
            "ncsq": np.ascontiguousarray(ncsq, np.float32),
            "ncsk": np.ascontiguousarray(ncsk, np.float32),
            "ncsv": np.ascontiguousarray(ncsv, np.float32),
            "abias": ab_t,
            "wout": np.ascontiguousarray(w_out[sl_q, :].astype(BF)),
        }
        if has_bias:
            im["bqr"] = np.ascontiguousarray(bq_c, np.float32)
            im["bkr"] = np.ascontiguousarray(bk_c, np.float32)
            im["bvr"] = np.ascontiguousarray(bv_c, np.float32)
        if fm:
            im["m2"] = np.ascontiguousarray(m2_t)
        in_maps.append(im)
    return in_maps, fm, has_bias


def kernel(x, input_mask, alibi, norm_w, norm_b, w_qkv, b_qkv, w_out, b_out):
    from concourse.bass_utils import run_bass_kernel_spmd

    in_maps, fm, has_bias = _shard_inputs(
        np.asarray(x), np.asarray(input_mask), np.asarray(alibi),
        np.asarray(norm_w), np.asarray(norm_b), np.asarray(w_qkv),
        np.asarray(b_qkv), np.asarray(w_out), np.asarray(b_out),
    )
    global _LAST_CFG
    _LAST_CFG = (fm, has_bias)
    nc = _get_nc(fm, has_bias)
    res = run_bass_kernel_spmd(nc, in_maps, core_ids=list(range(NCORES)))
    acc = res.results[0]["out"].astype(np.float32).copy()
    for c in range(1, NCORES):
        acc += res.results[c]["out"]
    acc += np.asarray(b_out, np.float32)[None, :]
    return acc.reshape(B, S, H)


# revision 19
# speedup vs baseline: 1.1178x; 1.0330x over previous
"""DeepSpeed-style self-attention block on 8 Trainium2 NeuronCores.

Tensor-parallel over heads (4 heads/core), DeepSpeed mp_size=8 style:
  - w_qkv column-sharded [H, 3H/8] (per-core wq/wk/wv [H, 512], bf16)
  - w_out row-sharded   [H/8, H]  -> per-core partial outputs
  - layernorm stats computed on device; partial-sum reduction + b_out on host.

Device kernel structure (per core, identical SPMD program, sharded inputs):
  Phase A: the host passes x twice in bf16 - row-major (for LN stats) and
    pre-transposed xT [H, T] (pure layout change).  Per 512-token chunk:
    DVE bn_stats/bn_aggr give mean/rstd/std columns; a tiny PE matmul
    transposes them to rows.  The QKV gemms run directly on RAW xT
    (bf16, full rate); layernorm is folded in algebraically:
       qT = ((W^T xT) - csum(W) (x) mu + b (x) std) * rstd[t]
    The two rank-1 terms are extra K=1 matmuls accumulated into the same
    PSUM; the per-token rstd scale is one DVE multiply against a
    PE-broadcast rstd row (q/k outputs, free-dim tokens) or a per-partition
    tensor_scalar (v output, partition-dim tokens).  norm_w / the
    1/sqrt(sqrt(hd)) scale are folded into the bf16 weights on the host;
    column sums are taken over the *rounded* bf16 weights so the mean
    correction is exact.  Weights stream as one 4MB slab DMA per
    (weight, chunk).  qT/kT [d, tok] and v [tok, d] go to DRAM scratch in
    bf16.
  Phase B: per (batch, head): scoresT = kT^T @ qT -> [k, q] psum blocks
    (two 512-wide kj blocks per 2-bank psum tile); causal handled as
    additive -50 tiles on the diagonal-straddling blocks only (DVE);
    exp via ACT with per-key (mask+alibi) bias; pairs of e tiles are
    pre-added on DVE so the row-sum ones-matmul runs once per 2 kj;
    PV matmul accumulates ctxT [d, q]; 1/sum broadcast via K=1 matmul,
    applied on the PSUM->SBUF ctx write (DVE).
    Rows that DeepSpeed's tied -10000 constants make attend over the
    *whole* sequence (all their visible keys input-masked) can only live
    among the first <128 queries (host-verified); a narrow pass-2 covers
    queries 0..127 over kj 4..15 with multiplicative key weights
    m2 = exp(alibi + maskbias + NEG), accumulated into the same psum.
    (-50 instead of DeepSpeed's -10000 keeps exp() in fp32 range; softmax
    is shift-invariant and masked weights come out ~1e-19, matching the
    reference's exact 0s well below tolerance.)
  Phase C: out-proj partial: out += ctxT^T @ w_out_shard per token tile.

All big matmuls run in bf16 (full rate); accumulation is f32 PSUM.
The walrus build here allows only ONE semaphore wait per instruction;
PatchedTileContext splits surplus Tile-emitted waits onto NoOps.
"""

import numpy as np
import ml_dtypes

import concourse.bass as bass
import concourse.mybir as mybir
import concourse.tile as tile
from concourse import masks

f32 = mybir.dt.float32
f32r = mybir.dt.float32r
bf16 = mybir.dt.bfloat16
BF = ml_dtypes.bfloat16

B, S, H, NH = 2, 2048, 4096, 32
HD = H // NH            # 128 head dim
NCORES = 8
HPC = NH // NCORES      # 4 heads per core
FPC = HPC * HD          # 512 sharded features per core
T = B * S               # 4096 tokens
KT = H // 128           # 32 contraction tiles
CHUNK = 512             # tokens per QKV chunk
NCHUNK = T // CHUNK     # 8
QTILE = 512             # query block in attention
NP2 = 12                # pass-2 kj blocks (kj 4..15) for fully-masked rows
LN_EPS = 1e-5
NEG = -50.0             # soft mask value (see module docstring)


class PatchedTileContext(tile.TileContext):
    """This container's walrus build rejects >1 sync-wait per instruction;
    split surplus waits onto preceding same-engine NoOps."""

    _wsplit_n = 0

    def _commit_instruction(self, inst, lazy_reg_writes: bool = True):
        si = inst.sync_info
        if si is not None and si.on_wait and len(si.on_wait) > 1:
            waits = list(si.on_wait)
            inst.sync_info = mybir.SyncInfo(
                on_wait=[waits[-1]], on_update=list(si.on_update or [])
            )
            for w in waits[:-1]:
                type(self)._wsplit_n += 1
                n = mybir.InstNoOp(name=f"wsplit-{type(self)._wsplit_n}")
                n.engine = inst.engine
                n.sync_info = mybir.SyncInfo(on_wait=[w], on_update=[])
                self._add_instruction(n)
        return super()._commit_instruction(inst, lazy_reg_writes)

    def _drain_and_barrier(self, tick_clock, wait_clock):
        from concourse.vector_clock import ScopedClock

        nc = self.nc
        collector = nc.sync.nop(nofuse=True)
        wait_clock.add_sem_waits(
            collector.ins, ScopedClock({None: tick_clock.global_clock})
        )
        waits = list(collector.ins.sync_info.on_wait)
        collector.ins.sync_info = mybir.SyncInfo(on_wait=[], on_update=[])
        for w in waits:
            n = nc.sync.nop(nofuse=True)
            n.ins.sync_info = mybir.SyncInfo(on_wait=[w], on_update=[])
        nc.sync.drain()
        nc.all_engine_barrier()
        assert self.sems is not None
        popped = nc._tile_sem_poison_stack.pop()
        assert popped is self._sem_poison
        nc.clear_and_free_semaphores(list(self.sems.allocated().values()))
        nc.all_engine_barrier()


AF = mybir.ActivationFunctionType


def build_nc(fm: bool = True, has_bias: bool = False):
    nc = bass.Bass(target_bir_lowering=False)

    xr = nc.declare_dram_parameter("xr", [T, H], bf16, isOutput=False).ap()
    xt = nc.declare_dram_parameter("xt", [H, T], bf16, isOutput=False).ap()
    wq = nc.declare_dram_parameter("wq", [H, FPC], bf16, isOutput=False).ap()
    wk = nc.declare_dram_parameter("wk", [H, FPC], bf16, isOutput=False).ap()
    wv = nc.declare_dram_parameter("wv", [H, FPC], bf16, isOutput=False).ap()
    # negated column sums of the (bf16-rounded) weights, and bias rows
    ncsq = nc.declare_dram_parameter("ncsq", [1, FPC], f32r, isOutput=False).ap()
    ncsk = nc.declare_dram_parameter("ncsk", [1, FPC], f32r, isOutput=False).ap()
    ncsv = nc.declare_dram_parameter("ncsv", [1, FPC], f32r, isOutput=False).ap()
    if has_bias:
        bqr = nc.declare_dram_parameter("bqr", [1, FPC], f32r, isOutput=False).ap()
        bkr = nc.declare_dram_parameter("bkr", [1, FPC], f32r, isOutput=False).ap()
        bvr = nc.declare_dram_parameter("bvr", [1, FPC], f32r, isOutput=False).ap()
    abias = nc.declare_dram_parameter(
        "abias", [128, B * HPC, S // 128], f32, isOutput=False
    ).ap()
    if fm:
        # multiplicative key weights exp(abias + NEG) for the narrow pass-2
        m2 = nc.declare_dram_parameter(
            "m2", [128, B * HPC, NP2], f32, isOutput=False
        ).ap()
    wout = nc.declare_dram_parameter("wout", [FPC, H], bf16, isOutput=False).ap()
    out = nc.declare_dram_parameter("out", [T, H], f32, isOutput=True).ap()

    # DRAM scratch
    qT_s = nc.dram_tensor("qT_s", [HPC, 128, T], bf16).ap()
    kT_s = nc.dram_tensor("kT_s", [HPC, 128, T], bf16).ap()
    v_s = nc.dram_tensor("v_s", [T, FPC], bf16).ap()

    xt_r = xt.rearrange("(kt p) t -> p kt t", p=128)
    w_r = {
        "q": wq.rearrange("(kt p) f -> p kt f", p=128),
        "k": wk.rearrange("(kt p) f -> p kt f", p=128),
        "v": wv.rearrange("(kt p) f -> p kt f", p=128),
    }

    with PatchedTileContext(nc) as tc:
        with tc.tile_pool(name="singles", bufs=1) as singles:
            identity = singles.tile([128, 128], f32)
            masks.make_identity(nc, identity[:])
            ones_f = singles.tile([128, 128], f32)
            nc.vector.memset(ones_f[:], 1.0)
            ones_r = singles.tile([128, 128], f32r)
            nc.scalar.activation(out=ones_r[:], in_=ones_f[:], func=AF.Copy)
            ones_bf = singles.tile([128, 1], bf16)
            with nc.allow_low_precision(reason="exact ones"):
                nc.vector.tensor_copy(out=ones_bf[:], in_=ones_f[:, 0:1])
            eps_t = singles.tile([128, 1], f32)
            nc.vector.memset(eps_t[:], LN_EPS)
            # additive causal tiles, one per diagonal offset d = (k0-q0)/128
            causal = singles.tile([128, 4, QTILE], f32)
            nc.gpsimd.memset(causal[:], 0.0)
            for d in range(4):
                nc.gpsimd.affine_select(
                    out=causal[:, d, :],
                    in_=causal[:, d, :],
                    compare_op=mybir.AluOpType.is_ge,
                    fill=NEG,
                    base=-(128 * d),
                    pattern=[[1, QTILE]],
                    channel_multiplier=-1,
                )
            # row constants (csum / bias rows)
            ncs_c = {}
            for nm, src in (("q", ncsq), ("k", ncsk), ("v", ncsv)):
                t = singles.tile([1, FPC], f32r, name=f"ncs_{nm}")
                nc.sync.dma_start(out=t[:], in_=src)
                ncs_c[nm] = t
            b_c = {}
            if has_bias:
                for nm, bsrc in (("q", bqr), ("k", bkr), ("v", bvr)):
                    t = singles.tile([65, FPC], f32r, name=f"b_{nm}")
                    nc.sync.dma_start(out=t[64:65, :], in_=bsrc)
                    b_c[nm] = t
            ab_c = singles.tile([128, B * HPC, S // 128], f32)
            nc.sync.dma_start(out=ab_c[:], in_=abias)
            if fm:
                m2_c = singles.tile([128, B * HPC, NP2], f32)
                nc.sync.dma_start(out=m2_c[:], in_=m2)
                m2_cb = singles.tile([128, B * HPC, NP2], bf16)
                nc.scalar.activation(out=m2_cb[:], in_=m2_c[:], func=AF.Copy)

            # ---------------- Phase A: LN stats + QKV gemms ----------------
            with tc.tile_pool(name="xrp", bufs=4) as xrp, \
                 tc.tile_pool(name="statp", bufs=4) as statp, \
                 tc.tile_pool(name="mvp", bufs=8) as mvp, \
                 tc.tile_pool(name="srowp", bufs=2) as srowp, \
                 tc.tile_pool(name="rssp", bufs=2) as rssp, \
                 tc.tile_pool(name="xtp", bufs=2) as xtp, \
                 tc.tile_pool(name="wp", bufs=2) as wp, \
                 tc.tile_pool(name="stp", bufs=6) as stp, \
                 tc.tile_pool(name="strp", bufs=1, space="PSUM") as strp, \
                 tc.tile_pool(name="rsbp", bufs=2, space="PSUM") as rsbp, \
                 tc.tile_pool(name="qpp", bufs=5, space="PSUM") as qpp:

                def emit_stats(c):
                    """LN stats for chunk c: returns (srow, rsb, mv tiles).
                    srow [65, CHUNK] f32r rows at PE-legal base partitions:
                    0=mean, 32=rstd, 64=std.
                    rsb = rstd broadcast [128, CHUNK] (PSUM f32)."""
                    statrow = strp.tile([65, CHUNK], f32, tag="statrow")
                    mvs = []
                    for tt in range(CHUNK // 128):
                        g = c * (CHUNK // 128) + tt
                        xr_t = xrp.tile([128, H], bf16)
                        nc.sync.dma_start(
                            out=xr_t[:], in_=xr[g * 128:(g + 1) * 128, :]
                        )
                        stats = statp.tile([128, H // 512, 6], f32)
                        xg = xr_t[:].rearrange("p (n f) -> p n f", f=512)
                        for n in range(H // 512):
                            nc.vector.bn_stats(out=stats[:, n, :], in_=xg[:, n, :])
                        mv = mvp.tile([128, 3], f32)
                        nc.vector.bn_aggr(out=mv[:, 0:2], in_=stats[:])
                        nc.scalar.activation(
                            out=mv[:, 2:3], in_=mv[:, 1:2], func=AF.Sqrt,
                            bias=eps_t[:], scale=1.0,
                        )
                        nc.vector.reciprocal(out=mv[:, 1:2], in_=mv[:, 2:3])
                        for i in range(3):
                            nc.tensor.matmul(
                                statrow[
                                    32 * i:32 * i + 1,
                                    tt * 128:(tt + 1) * 128,
                                ],
                                lhsT=mv[:, i:i + 1], rhs=identity[:],
                                start=True, stop=True,
                            )
                        mvs.append(mv)
                    srow = srowp.tile([65, CHUNK], f32r, tag="srow")
                    with nc.allow_low_precision(reason="f32r matmul operand"):
                        nc.vector.tensor_copy(out=srow[:], in_=statrow[:])
                    rsb = rsbp.tile([128, CHUNK], f32, tag="rsb")
                    nc.tensor.matmul(
                        rsb[:], lhsT=ones_r[32:33, :], rhs=srow[32:33, :],
                        start=True, stop=True,
                    )
                    rsbs = rssp.tile([128, CHUNK], f32, tag="rsbs")
                    nc.scalar.activation(out=rsbs[:], in_=rsb[:], func=AF.Copy)
                    return srow, rsbs, mvs

                def load_slab(nm, pieces=2):
                    ws = wp.tile([128, KT, FPC], bf16, tag="wslab")
                    step = KT // pieces
                    for p in range(pieces):
                        nc.sync.dma_start(
                            out=ws[:, p * step:(p + 1) * step, :],
                            in_=w_r[nm][:, p * step:(p + 1) * step, :],
                        )
                    return ws

                def gemm_block(nm, ws, xt_t, srow, rsb, mvs, c0):
                    flip = nm in ("q", "k")
                    pss = [
                        qpp.tile([128, CHUNK], f32, tag="pss", name=f"pss{f}")
                        for f in range(4)
                    ]
                    for kt in range(KT):
                        for f in range(4):
                            fs = slice(f * 128, (f + 1) * 128)
                            if flip:
                                nc.tensor.matmul(
                                    pss[f][:], lhsT=ws[:, kt, fs],
                                    rhs=xt_t[:, kt, :],
                                    start=(kt == 0), stop=False,
                                )
                            else:
                                nc.tensor.matmul(
                                    pss[f][:], lhsT=xt_t[:, kt, fs],
                                    rhs=ws[:, kt, :],
                                    start=(kt == 0), stop=False,
                                )
                    for f in range(4):
                        fs = slice(f * 128, (f + 1) * 128)
                        if flip:
                            # psum[d, tok] -= csum[d] * mu[tok] (+ b*std)
                            if has_bias:
                                nc.tensor.matmul(
                                    pss[f][:], lhsT=b_c[nm][64:65, fs],
                                    rhs=srow[64:65, :], start=False,
                                    stop=False,
                                )
                            nc.tensor.matmul(
                                pss[f][:], lhsT=ncs_c[nm][0:1, fs],
                                rhs=srow[0:1, :], start=False, stop=True,
                            )
                        else:
                            if has_bias:
                                nc.tensor.matmul(
                                    pss[f][:], lhsT=srow[64:65, fs],
                                    rhs=b_c[nm][64:65, :], start=False,
                                    stop=False,
                                )
                            nc.tensor.matmul(
                                pss[f][:], lhsT=srow[0:1, fs],
                                rhs=ncs_c[nm][0:1, :], start=False, stop=True,
                            )
                    dst = {"q": qT_s, "k": kT_s}.get(nm)
                    for f in range(4):
                        st = stp.tile([128, CHUNK], bf16, tag="st", name=f"st{f}")
                        with nc.allow_low_precision(reason="bf16 scratch"):
                            if flip:
                                nc.vector.tensor_mul(
                                    out=st[:], in0=pss[f][:], in1=rsb[:]
                                )
                                nc.sync.dma_start(
                                    out=dst[f, :, c0:c0 + CHUNK], in_=st[:]
                                )
                            else:
                                nc.vector.tensor_scalar_mul(
                                    out=st[:], in0=pss[f][:],
                                    scalar1=mvs[f][:, 1:2],
                                )
                                nc.sync.dma_start(
                                    out=v_s[c0 + f * 128:c0 + (f + 1) * 128, :],
                                    in_=st[:],
                                )

                # prologue: wq/xT quarter DMAs first (first gemms gate
                # on them; rank-1s now come last so LN stats have a whole
                # weight-pass of slack), x-tile loads interleave behind.
                xt_ts = {}
                xt_ts[0] = xtp.tile(
                    [128, KT, CHUNK], bf16, tag="xts", name="xts0"
                )
                slab = {"q": wp.tile([128, KT, FPC], bf16, tag="wslab",
                                     name="wq0")}
                for p in range(4):
                    st8 = slice(p * (KT // 4), (p + 1) * (KT // 4))
                    nc.sync.dma_start(
                        out=slab["q"][:, st8, :], in_=w_r["q"][:, st8, :]
                    )
                    nc.sync.dma_start(
                        out=xt_ts[0][:, st8, :],
                        in_=xt_r[:, st8, 0:CHUNK],
                    )
                stat = {0: emit_stats(0)}
                for c in range(NCHUNK):
                    c0 = c * CHUNK
                    if c + 1 < NCHUNK:
                        xt_ts[c + 1] = xtp.tile(
                            [128, KT, CHUNK], bf16, tag="xts",
                            name=f"xts{c + 1}",
                        )
                        for p in range(2):
                            sth = slice(p * (KT // 2), (p + 1) * (KT // 2))
                            nc.sync.dma_start(
                                out=xt_ts[c + 1][:, sth, :],
                                in_=xt_r[:, sth, c0 + CHUNK:c0 + 2 * CHUNK],
                            )
                    slab["k"] = load_slab("k")
                    if c + 1 < NCHUNK:
                        stat[c + 1] = emit_stats(c + 1)
                    srow, rsb, mvs = stat.pop(c)
                    xt_t = xt_ts.pop(c)
                    gemm_block("q", slab.pop("q"), xt_t, srow, rsb, mvs, c0)
                    slab["v"] = load_slab("v")
                    gemm_block("k", slab.pop("k"), xt_t, srow, rsb, mvs, c0)
                    if c + 1 < NCHUNK:
                        slab["q"] = load_slab("q")
                    gemm_block("v", slab.pop("v"), xt_t, srow, rsb, mvs, c0)

            # ------------- Phase B: attention -------------
            with tc.tile_pool(name="ctxp", bufs=1) as ctxp:
                ctx_t = [
                    ctxp.tile([128, S], bf16, tag=f"ctx{u}", name=f"ctx{u}")
                    for u in range(B * HPC)
                ]
                with tc.tile_pool(name="qtp", bufs=2) as qtp, \
                     tc.tile_pool(name="ktp", bufs=2) as ktp, \
                     tc.tile_pool(name="vp", bufs=2) as vp, \
                     tc.tile_pool(name="ep", bufs=6) as ep, \
                     tc.tile_pool(name="esp", bufs=6) as esp, \
                     tc.tile_pool(name="enp", bufs=2) as enp, \
                     tc.tile_pool(name="vnp", bufs=4) as vnp, \
                     tc.tile_pool(name="rp", bufs=3) as rp, \
                     tc.tile_pool(name="scp", bufs=3, space="PSUM") as scp, \
                     tc.tile_pool(name="cpp", bufs=2, space="PSUM") as cpp, \
                     tc.tile_pool(name="smp", bufs=2, space="PSUM") as smp, \
                     tc.tile_pool(name="mscp", bufs=1, space="PSUM") as mscp:
                    for u in range(B * HPC):
                        b, hh = divmod(u, HPC)
                        qt = qtp.tile([128, S], bf16)
                        nc.sync.dma_start(
                            out=qt[:], in_=qT_s[hh, :, b * S:(b + 1) * S]
                        )
                        kt_h = ktp.tile([128, S], bf16)
                        nc.sync.dma_start(
                            out=kt_h[:], in_=kT_s[hh, :, b * S:(b + 1) * S]
                        )
                        vt = vp.tile([128, S // 128, 128], bf16)
                        nc.sync.dma_start(
                            out=vt[:],
                            in_=v_s[
                                b * S:(b + 1) * S, hh * 128:(hh + 1) * 128
                            ].rearrange("(kj p) d -> p kj d", p=128),
                        )
                        # producer/consumer queue: consumers (es add, sums,
                        # PV, finalize) trail the score/exp producers by
                        # DEPTH entries so PE never waits on ACT in program
                        # order.
                        pend = []

                        def drain(n, pend=pend):
                            while len(pend) > n:
                                pend.pop(0)()

                        for qi in range(S // QTILE):
                            q0 = qi * QTILE
                            nkj = (q0 + QTILE) // 128
                            ctx_ps = cpp.tile(
                                [128, QTILE], f32, tag="ctxps",
                                name=f"ctxps{u}_{qi}",
                            )
                            sums = smp.tile(
                                [1, QTILE], f32, tag="sums",
                                name=f"sums{u}_{qi}",
                            )
                            ext = fm and qi == 0
                            for kjp in range(nkj // 2):
                                es2 = []
                                ws2 = []
                                for j in range(2):
                                    kj = 2 * kjp + j
                                    sc = scp.tile(
                                        [128, QTILE], f32, tag="sc",
                                        name=f"sc{u}_{qi}_{kj}",
                                    )
                                    nc.tensor.matmul(
                                        sc[:],
                                        lhsT=kt_h[:, kj * 128:(kj + 1) * 128],
                                        rhs=qt[:, q0:q0 + QTILE],
                                        start=True, stop=True,
                                    )
                                    d = kj - (q0 // 128)
                                    w0 = 128 * d if d > 0 else 0
                                    if d >= 0:
                                        # triangle add on the straddling
                                        # 128-col block only
                                        nc.vector.tensor_add(
                                            out=sc[:, 128 * d:128 * (d + 1)],
                                            in0=sc[:, 128 * d:128 * (d + 1)],
                                            in1=causal[
                                                :, d,
                                                128 * d:128 * (d + 1),
                                            ],
                                        )
                                    e = ep.tile(
                                        [128, QTILE], bf16, tag="e",
                                        name=f"e{u}_{qi}_{kj}",
                                    )
                                    nc.scalar.activation(
                                        out=e[:, w0:], in_=sc[:, w0:],
                                        func=AF.Exp,
                                        bias=ab_c[:, u, kj:kj + 1], scale=1.0,
                                    )
                                    es2.append(e)
                                    ws2.append(w0)

                                def consume(kjp=kjp, es2=es2, ws2=ws2,
                                            ctx_ps=ctx_ps, sums=sums, vt=vt,
                                            last=(kjp == nkj // 2 - 1)
                                            and not ext):
                                    w0, w1 = ws2
                                    es = esp.tile(
                                        [128, QTILE], bf16, tag="es"
                                    )
                                    if kjp == 0 and w1 > 0:
                                        # first pair of the bank: zero e1's
                                        # invalid strip so one full-width
                                        # start=True matmul covers the bank
                                        nc.vector.memset(
                                            es2[1][:, 0:w1], 0.0
                                        )
                                        w1 = 0
                                    with nc.allow_low_precision(
                                        reason="e pair add"
                                    ):
                                        nc.vector.tensor_add(
                                            out=es[:, w1:],
                                            in0=es2[0][:, w1:],
                                            in1=es2[1][:, w1:],
                                        )
                                    nc.tensor.matmul(
                                        sums[0:1, w1:], lhsT=ones_bf[:],
                                        rhs=es[:, w1:],
                                        start=(kjp == 0), stop=last,
                                        skip_group_check=True,
                                    )
                                    if w1 > w0:
                                        nc.tensor.matmul(
                                            sums[0:1, w0:w1],
                                            lhsT=ones_bf[:],
                                            rhs=es2[0][:, w0:w1],
                                            start=False, stop=last,
                                            skip_group_check=True,
                                        )
                                    nc.tensor.matmul(
                                        ctx_ps[:, w0:], lhsT=vt[:, 2 * kjp, :],
                                        rhs=es2[0][:, w0:],
                                        start=(kjp == 0), stop=False,
                                        skip_group_check=True,
                                    )
                                    nc.tensor.matmul(
                                        ctx_ps[:, w1:],
                                        lhsT=vt[:, 2 * kjp + 1, :],
                                        rhs=es2[1][:, w1:],
                                        start=False, stop=last,
                                        skip_group_check=True,
                                    )

                                pend.append(consume)
                                drain(2)
                            if ext:
                                # narrow pass-2: queries 0..127, kj 4..15,
                                # multiplicative key weights m2
                                for g in range(NP2 // 4):
                                    scn = mscp.tile(
                                        [128, 4, 128], f32, tag="msc",
                                        name=f"scn{u}_{g}",
                                    )
                                    en = enp.tile(
                                        [128, 4, 128], bf16, tag="en",
                                        name=f"en{u}_{g}",
                                    )
                                    for j in range(4):
                                        kj = 4 + g * 4 + j
                                        nc.tensor.matmul(
                                            scn[:, j, :],
                                            lhsT=kt_h[
                                                :, kj * 128:(kj + 1) * 128
                                            ],
                                            rhs=qt[:, 0:128],
                                            start=True, stop=True,
                                        )
                                    nc.scalar.activation(
                                        out=en[:], in_=scn[:], func=AF.Exp,
                                        scale=1.0,
                                    )

                                    def consume2(g=g, en=en, vt=vt, u=u,
                                                 ctx_ps=ctx_ps, sums=sums):
                                        for j in range(4):
                                            kj = 4 + g * 4 + j
                                            ki = g * 4 + j
                                            vtn = vnp.tile(
                                                [128, 128], bf16, tag="vtn"
                                            )
                                            with nc.allow_low_precision(
                                                reason="bf16 v scale"
                                            ):
                                                nc.vector.tensor_scalar_mul(
                                                    out=vtn[:],
                                                    in0=vt[:, kj, :],
                                                    scalar1=m2_c[
                                                        :, u, ki:ki + 1
                                                    ],
                                                )
                                            last2 = ki == NP2 - 1
                                            nc.tensor.matmul(
                                                sums[0:1, 0:128],
                                                lhsT=m2_cb[:, u, ki:ki + 1],
                                                rhs=en[:, j, :],
                                                start=False, stop=last2,
                                                skip_group_check=True,
                                            )
                                            nc.tensor.matmul(
                                                ctx_ps[:, 0:128], lhsT=vtn[:],
                                                rhs=en[:, j, :],
                                                start=False, stop=last2,
                                                skip_group_check=True,
                                            )

                                    pend.append(consume2)
                                    drain(2)

                            def finalize(u=u, qi=qi, q0=q0, ctx_ps=ctx_ps,
                                         sums=sums):
                                rcp = rp.tile([1, QTILE], f32r, tag="rcp")
                                with nc.allow_low_precision(
                                    reason="f32r matmul operand"
                                ):
                                    nc.vector.reciprocal(
                                        out=rcp[:], in_=sums[:]
                                    )
                                rsb2 = mscp.tile(
                                    [128, QTILE], f32, tag="msc",
                                    name=f"rsb2{u}_{qi}",
                                )
                                nc.tensor.matmul(
                                    rsb2[:], lhsT=ones_r[0:1, :], rhs=rcp[:],
                                    start=True, stop=True,
                                )
                                rsbs2 = rp.tile(
                                    [128, QTILE], f32, tag="rsbs2"
                                )
                                nc.vector.tensor_copy(
                                    out=rsbs2[:], in_=rsb2[:]
                                )
                                with nc.allow_low_precision(reason="bf16 ctx"):
                                    nc.vector.tensor_mul(
                                        out=ctx_t[u][:, q0:q0 + QTILE],
                                        in0=ctx_ps[:], in1=rsbs2[:],
                                    )

                            pend.append(finalize)
                            drain(3)
                        drain(0)

                # ------------- Phase C: out-proj -------------
                with tc.tile_pool(name="wop", bufs=2) as wop, \
                     tc.tile_pool(name="osp", bufs=6) as osp, \
                     tc.tile_pool(name="opp", bufs=4, space="PSUM") as opp:
                    for hs in range(H // 512):
                        wo_t = wop.tile([128, HPC, 512], bf16)
                        nc.sync.dma_start(
                            out=wo_t[:],
                            in_=wout[:, hs * 512:(hs + 1) * 512].rearrange(
                                "(f p) h -> p f h", p=128
                            ),
                        )
                        for ti in range(T // 128):
                            bb, tloc = divmod(ti, S // 128)
                            ps = opp.tile([128, 512], f32, tag="ops")
                            for f in range(HPC):
                                nc.tensor.matmul(
                                    ps[:],
                                    lhsT=ctx_t[bb * HPC + f][
                                        :, tloc * 128:(tloc + 1) * 128
                                    ],
                                    rhs=wo_t[:, f, :],
                                    start=(f == 0), stop=(f == HPC - 1),
                                )
                            ost = osp.tile([128, 512], f32, tag="ost")
                            nc.scalar.activation(
                                out=ost[:], in_=ps[:], func=AF.Copy
                            )
                            nc.sync.dma_start(
                                out=out[
                                    ti * 128:(ti + 1) * 128,
                                    hs * 512:(hs + 1) * 512,
                                ],
                                in_=ost[:],
                            )
    return nc


_NC_CACHE = {}
_LAST_CFG = (True, False)


def _get_nc(fm: bool = True, has_bias: bool = False):
    key = (fm, has_bias)
    if key not in _NC_CACHE:
        _NC_CACHE[key] = build_nc(fm, has_bias)
    return _NC_CACHE[key]


def _shard_inputs(x, input_mask, alibi, norm_w, norm_b, w_qkv, b_qkv, w_out, b_out):
    scale = np.float32(1.0 / np.sqrt(np.sqrt(np.float32(HD))))
    xf = np.ascontiguousarray(x.reshape(T, H), dtype=np.float32)
    x_bf = xf.astype(BF)
    xT_bf = np.ascontiguousarray(x_bf.T)
    nw = norm_w.astype(np.float32)
    nb = norm_b.astype(np.float32)
    mask_bias = (1.0 - input_mask.astype(np.float32)) * np.float32(NEG)  # [B, S]

    # fully-masked rows live only among the first lz = leading-masked-key
    # count queries; the narrow pass-2 covers queries 0..127.
    lz = 0
    for b in range(B):
        mb = input_mask[b]
        lz = max(lz, int(np.argmax(mb == 1)) if mb.any() else S)
    assert lz < 128, f"leading masked-key run {lz} >= 128 unsupported"
    fm = lz > 0

    in_maps = []
    for c in range(NCORES):
        sl_q = slice(c * FPC, (c + 1) * FPC)
        sl_k = slice(H + c * FPC, H + (c + 1) * FPC)
        sl_v = slice(2 * H + c * FPC, 2 * H + (c + 1) * FPC)
        wq_c = ((nw[:, None] * w_qkv[:, sl_q]) * scale).astype(BF)
        wk_c = ((nw[:, None] * w_qkv[:, sl_k]) * scale).astype(BF)
        wv_c = (nw[:, None] * w_qkv[:, sl_v]).astype(BF)
        # negated column sums over the *rounded* weights (exact mean fold)
        ncsq = -wq_c.astype(np.float32).sum(0)[None, :]
        ncsk = -wk_c.astype(np.float32).sum(0)[None, :]
        ncsv = -wv_c.astype(np.float32).sum(0)[None, :]
        bq_c = ((b_qkv[sl_q] + nb @ w_qkv[:, sl_q]) * scale)[None, :]
        bk_c = ((b_qkv[sl_k] + nb @ w_qkv[:, sl_k]) * scale)[None, :]
        bv_c = (b_qkv[sl_v] + nb @ w_qkv[:, sl_v])[None, :]
        has_bias = bool(
            np.abs(bq_c).max() > 0 or np.abs(bk_c).max() > 0
            or np.abs(bv_c).max() > 0
        )
        ab = np.empty((B * HPC, S), np.float32)
        for b in range(B):
            for hh in range(HPC):
                ab[b * HPC + hh] = alibi[c * HPC + hh, 0, :] + mask_bias[b]
        ab_t = np.ascontiguousarray(
            ab.reshape(B * HPC, S // 128, 128).transpose(2, 0, 1)
        )
        m2_t = np.exp(ab_t[:, :, 4:16].astype(np.float64) + NEG).astype(np.float32)
        im = {
            "xr": x_bf,
            "xt": xT_bf,
            "wq": np.ascontiguousarray(wq_c),
            "wk": np.ascontiguousarray(wk_c),
            "wv": np.ascontiguousarray(wv_c),
            "ncsq": np.ascontiguousarray(ncsq, np.float32),
            "ncsk": np.ascontiguousarray(ncsk, np.float32),
            "ncsv": np.ascontiguousarray(ncsv, np.float32),
            "abias": ab_t,
            "wout": np.ascontiguousarray(w_out[sl_q, :].astype(BF)),
        }
        if has_bias:
            im["bqr"] = np.ascontiguousarray(bq_c, np.float32)
            im["bkr"] = np.ascontiguousarray(bk_c, np.float32)
            im["bvr"] = np.ascontiguousarray(bv_c, np.float32)
        if fm:
            im["m2"] = np.ascontiguousarray(m2_t)
        in_maps.append(im)
    return in_maps, fm, has_bias


def kernel(x, input_mask, alibi, norm_w, norm_b, w_qkv, b_qkv, w_out, b_out):
    from concourse.bass_utils import run_bass_kernel_spmd

    in_maps, fm, has_bias = _shard_inputs(
        np.asarray(x), np.asarray(input_mask), np.asarray(alibi),
        np.asarray(norm_w), np.asarray(norm_b), np.asarray(w_qkv),
        np.asarray(b_qkv), np.asarray(w_out), np.asarray(b_out),
    )
    global _LAST_CFG
    _LAST_CFG = (fm, has_bias)
    nc = _get_nc(fm, has_bias)
    res = run_bass_kernel_spmd(nc, in_maps, core_ids=list(range(NCORES)))
    acc = res.results[0]["out"].astype(np.float32).copy()
    for c in range(1, NCORES):
        acc += res.results[c]["out"]
    acc += np.asarray(b_out, np.float32)[None, :]
    return acc.reshape(B, S, H)
